# revision 6
# baseline (speedup 1.0000x reference)
"""Trainium2 Bass kernel for the SLAYER-style 2-layer spiking encoder.

Reference computation (per time step t, batch b):
    x:  (T=500, B=16, F=6300) binary spikes
    p1 = alpha_psp(x)            # linear 2-state recurrence per feature
    u1 = p1 @ w1.T               # (T,B,1024)
    s1 = spike_dyn(u1)           # nonlinear threshold + refractory, per neuron
    p2 = alpha_psp(s1)
    u2 = p2 @ w2.T               # (T,B,20)
    s2 = spike_dyn(u2)
    out = s2 transposed to (B, 20, T)

Key algebraic facts exploited here:
  * alpha_psp is a linear time-invariant per-feature filter, so it commutes
    with the (feature-contracting) matmuls:  alpha_psp(x) @ W == alpha_psp(x @ W).
    We therefore matmul the *raw binary spikes* (exact in bf16) and run the
    alpha filter on the (T,1024) post-matmul signal instead of (T,6300).
  * alpha_psp's two cascaded one-pole recurrences map directly onto the DVE
    `tensor_tensor_scan` instruction (one instruction per 500-step scan):
        p[t] = d*p[t-1] + v[t]
        r[t] = d*r[t-1] + p[t]          (then  c*q[t] = c*d*r[t-1])
  * spike_dyn is nonlinear (spike threshold feeding a refractory filter) and
    must be stepped sequentially, but its state decays by e^-1 per step, so
    chunks of the time axis can be processed in parallel SIMD lanes after a
    30-step zero-state warmup (state error ~1e-11, far below any decision
    margin).  Each step is 4 fused DVE `scalar_tensor_tensor` ops.

Sharding: pure data-parallel over batch: 8 cores x 2 batches.  No collectives.

Numerics: fc1/fc2 run in bf16 (inputs are exactly representable binary spikes;
weights are rounded).  The refractory scale cref is chosen bf16-representable
and used consistently, and the 1/cref is folded into the layer-2 scan scale,
so the only approximation vs the fp32 reference is the bf16 weight rounding.
"""

import numpy as np
import ml_dtypes

# ---------------------------------------------------------------- constants
B_TOT = 16
B_PER = 2
N_CORES = 8
T = 500
F_IN = 6300
F_PAD = 6400          # 50 k-tiles of 128
H1 = 1024             # fc1 output neurons -> 8 o-tiles of 128
H2 = 20               # fc2 output neurons
KT1 = F_PAD // 128    # 50
OT1 = H1 // 128       # 8
KT2 = H1 // 128       # 8

THETA = 10.0
D = float(np.float32(np.exp(-1.0)))        # decay per step
C = float(np.float32(np.e))                # alpha kernel scale (tau=1)
CD = C * D                                 # scale for u = c*q = c*d*r[t-1]
# refractory scale, snapped to the nearest bf16 so device-side bf16 stores of
# cref*s are exact; the inverse is folded into the layer-2 scan epilogue.
CREF = float(np.asarray(-2.0 * THETA * np.e, dtype=ml_dtypes.bfloat16).astype(np.float32))

# spike-chain time chunking: 10 chunks x 50 steps, 30-step warmup
NCH = 10
CHL = 50
WARM = 30
NSTEP = CHL + WARM    # 80

BF16 = ml_dtypes.bfloat16

_CACHE = {}


def _build():
    import concourse.bass as bass
    import concourse.bacc as bacc
    import concourse.mybir as mybir
    import concourse.tile as tile

    f32 = mybir.dt.float32
    bf16 = mybir.dt.bfloat16
    MULT = mybir.AluOpType.mult
    ADD = mybir.AluOpType.add
    IS_GE = mybir.AluOpType.is_ge
    COPY = mybir.ActivationFunctionType.Copy

    nc = bacc.Bacc("TRN2", target_bir_lowering=False, debug=False,
                   num_devices=N_CORES)

    x_d = nc.dram_tensor("x", [B_PER, F_PAD, T], bf16, kind="ExternalInput").ap()
    w1t_d = nc.dram_tensor("w1t", [F_PAD, H1], bf16, kind="ExternalInput").ap()
    w2t_d = nc.dram_tensor("w2t", [128, KT2 * H2], bf16, kind="ExternalInput").ap()
    y_d = nc.dram_tensor("y", [H2, B_PER * T], f32, kind="ExternalOutput").ap()

    with tile.TileContext(nc) as tc:
        with (
            tc.tile_pool(name="xres", bufs=2) as xres,
            tc.tile_pool(name="w1k", bufs=4) as w1kp,
            tc.tile_pool(name="wee", bufs=1) as wee,
            tc.tile_pool(name="ust", bufs=2) as ustp,
            tc.tile_pool(name="sst", bufs=2) as sstp,
            tc.tile_pool(name="scan", bufs=4) as scanp,
            tc.tile_pool(name="cst", bufs=1) as cstp,
            tc.tile_pool(name="state", bufs=6) as statep,
            tc.tile_pool(name="l2", bufs=1) as l2p,
            tc.tile_pool(name="ps", bufs=8, space="PSUM") as psp,
        ):
            # ---- constants
            dconst = cstp.tile([128, T], f32, tag="dconst")
            nc.gpsimd.memset(dconst[:], D)
            crefc = cstp.tile([128, NCH * OT1], f32, tag="crefc")
            nc.gpsimd.memset(crefc[:], CREF)
            cref2 = cstp.tile([H2, B_PER * NCH], f32, tag="cref2")
            nc.gpsimd.memset(cref2[:], CREF)

            # ---- w2 (tiny, resident)
            w2sb = wee.tile([128, KT2 * H2], bf16, tag="w2sb")
            nc.sync.dma_start(w2sb[:], w2t_d[:])

            # ---- x: resident, both batches
            xsb = []
            for b in range(B_PER):
                xt = xres.tile([128, KT1 * T], bf16, tag="xsb")
                xv = xt[:].rearrange("p (k t) -> p k t", k=KT1)
                src = x_d[b].rearrange("(k p) t -> p k t", p=128)
                for j in range(0, KT1, 10):
                    nc.sync.dma_start(xv[:, j:j + 10], src[:, j:j + 10])
                xsb.append(xv)

            ust, sst = [], []
            # ================= per-batch: fc1 + alpha scans + spike chain
            for b in range(B_PER):
                u_b = ustp.tile([128, OT1 * T], f32, tag="ust")
                s_b = sstp.tile([128, OT1 * T], bf16, tag="sst")
                ust.append(u_b)
                sst.append(s_b)

                # fc1: v1[o, t] += w1t[k, o] * x[k, t]
                v1 = [psp.tile([128, T], f32, tag="ps", name=f"v1_{b}_{ot}")
                      for ot in range(OT1)]
                for k in range(KT1):
                    w1k = w1kp.tile([128, H1], bf16, tag="w1k")
                    nc.sync.dma_start(w1k[:], w1t_d[k * 128:(k + 1) * 128, :])
                    for ot in range(OT1):
                        nc.tensor.matmul(
                            v1[ot][:],
                            w1k[:, ot * 128:(ot + 1) * 128],
                            xsb[b][:, k, :],
                            start=(k == 0), stop=(k == KT1 - 1),
                        )

                # alpha_psp scans + membrane precompute: U = c*d*r[t-1] - theta
                for ot in range(OT1):
                    p_t = scanp.tile([128, T], f32, tag="scan")
                    r_t = scanp.tile([128, T], f32, tag="scan")
                    nc.vector.tensor_tensor_scan(
                        p_t[:], dconst[:], v1[ot][:], 0.0, op0=MULT, op1=ADD)
                    nc.vector.tensor_tensor_scan(
                        r_t[:], dconst[:], p_t[:], 0.0, op0=MULT, op1=ADD)
                    nc.gpsimd.memset(u_b[:, ot * T:ot * T + 1], -THETA)
                    nc.scalar.activation(
                        u_b[:, ot * T + 1:(ot + 1) * T],
                        r_t[:, 0:T - 1], COPY, bias=-THETA, scale=CD)

                # spike chain, time-chunked:
                #   Z = d*Z + P ; M = d*Z + U_t ; G = (M>=0)*cref ; P = d*P + G
                z4 = statep.tile([128, OT1 * NCH], f32, tag="state")
                p4 = statep.tile([128, OT1 * NCH], f32, tag="state")
                m4 = statep.tile([128, OT1 * NCH], f32, tag="state")
                nc.gpsimd.memset(z4[:], 0.0)
                nc.gpsimd.memset(p4[:], 0.0)
                zv = z4[:].rearrange("p (g j) -> p g j", g=OT1)
                pv = p4[:].rearrange("p (g j) -> p g j", g=OT1)
                mv = m4[:].rearrange("p (g j) -> p g j", g=OT1)
                cv = crefc[:].rearrange("p (g j) -> p g j", g=OT1)
                u4 = u_b[:].rearrange("p (g j c) -> p g j c", g=OT1, j=NCH)
                s4 = s_b[:].rearrange("p (g j c) -> p g j c", g=OT1, j=NCH)

                for i in range(NSTEP):
                    if i < WARM:
                        slj, jj, ci = slice(1, NCH), slice(0, NCH - 1), i + CHL - WARM
                    else:
                        slj, jj, ci = slice(0, NCH), slice(0, NCH), i - WARM
                    zs, ps_, ms = zv[:, :, slj], pv[:, :, slj], mv[:, :, slj]
                    us, ss = u4[:, :, jj, ci], s4[:, :, jj, ci]
                    nc.vector.scalar_tensor_tensor(zs, zs, D, ps_, op0=MULT, op1=ADD)
                    nc.vector.scalar_tensor_tensor(ms, zs, D, us, op0=MULT, op1=ADD)
                    nc.vector.scalar_tensor_tensor(ss, ms, 0.0, cv[:, :, slj], op0=IS_GE, op1=MULT)
                    nc.vector.scalar_tensor_tensor(ps_, ps_, D, ss, op0=MULT, op1=ADD)

            # ================= layer 2 (both batches; PSUM banks free again)
            # batch lives on the FREE dim: [20, b*T + t] (partition base must be 0)
            u2 = l2p.tile([H2, B_PER * T], f32, tag="u2")
            s2 = l2p.tile([H2, B_PER * T], f32, tag="s2")
            for b in range(B_PER):
                v2 = psp.tile([H2, T], f32, tag="ps", name=f"v2_{b}")
                for kt in range(KT2):
                    nc.tensor.matmul(
                        v2[:],
                        w2sb[:, kt * H2:(kt + 1) * H2],
                        sst[b][:, kt * T:(kt + 1) * T],
                        start=(kt == 0), stop=(kt == KT2 - 1),
                    )
                p2t = scanp.tile([H2, T], f32, tag="scan")
                r2t = scanp.tile([H2, T], f32, tag="scan")
                nc.vector.tensor_tensor_scan(
                    p2t[:], dconst[0:H2, :], v2[:], 0.0, op0=MULT, op1=ADD)
                nc.vector.tensor_tensor_scan(
                    r2t[:], dconst[0:H2, :], p2t[:], 0.0, op0=MULT, op1=ADD)
                c0 = b * T
                nc.gpsimd.memset(u2[:, c0:c0 + 1], -THETA)
                nc.scalar.activation(
                    u2[:, c0 + 1:c0 + T], r2t[:, 0:T - 1], COPY,
                    bias=-THETA, scale=CD / CREF)

            # layer-2 spike chain (20p x 2b*10j lanes), same chunking; emits raw s2
            z2 = l2p.tile([H2, B_PER * NCH], f32, tag="z2")
            pp2 = l2p.tile([H2, B_PER * NCH], f32, tag="pp2")
            m2 = l2p.tile([H2, B_PER * NCH], f32, tag="m2")
            g2 = l2p.tile([H2, B_PER * NCH], f32, tag="g2")
            nc.gpsimd.memset(z2[:], 0.0)
            nc.gpsimd.memset(pp2[:], 0.0)
            z2v = z2[:].rearrange("p (b j) -> p b j", b=B_PER)
            pp2v = pp2[:].rearrange("p (b j) -> p b j", b=B_PER)
            m2v = m2[:].rearrange("p (b j) -> p b j", b=B_PER)
            g2v = g2[:].rearrange("p (b j) -> p b j", b=B_PER)
            c2v = cref2[:].rearrange("p (b j) -> p b j", b=B_PER)
            u24 = u2[:].rearrange("p (b j c) -> p b j c", b=B_PER, j=NCH)
            s24 = s2[:].rearrange("p (b j c) -> p b j c", b=B_PER, j=NCH)
            for i in range(NSTEP):
                if i < WARM:
                    slj, jj, ci = slice(1, NCH), slice(0, NCH - 1), i + CHL - WARM
                else:
                    slj, jj, ci = slice(0, NCH), slice(0, NCH), i - WARM
                zs, ps_ = z2v[:, :, slj], pp2v[:, :, slj]
                ms, gs = m2v[:, :, slj], g2v[:, :, slj]
                us, ss = u24[:, :, jj, ci], s24[:, :, jj, ci]
                nc.vector.scalar_tensor_tensor(zs, zs, D, ps_, op0=MULT, op1=ADD)
                nc.vector.scalar_tensor_tensor(ms, zs, D, us, op0=MULT, op1=ADD)
                nc.vector.tensor_scalar(ss, ms, 0.0, None, op0=IS_GE)
                nc.vector.scalar_tensor_tensor(gs, ms, 0.0, c2v[:, :, slj], op0=IS_GE, op1=MULT)
                nc.vector.scalar_tensor_tensor(ps_, ps_, D, gs, op0=MULT, op1=ADD)

            nc.sync.dma_start(y_d[:], s2[:])

    nc.compile()
    return nc


def _get_nc():
    if "nc" not in _CACHE:
        _CACHE["nc"] = _build()
    return _CACHE["nc"]


def kernel(downsampled: np.ndarray, w1: np.ndarray, w2: np.ndarray) -> np.ndarray:
    from concourse.bass_utils import run_bass_kernel_spmd

    nc = _get_nc()

    x = np.ascontiguousarray(downsampled.reshape(B_TOT, F_IN, T))
    xpad = np.zeros((B_TOT, F_PAD, T), dtype=BF16)
    xpad[:, :F_IN] = x.astype(BF16)

    w1t = np.zeros((F_PAD, H1), dtype=BF16)
    w1t[:F_IN] = np.ascontiguousarray(w1.T).astype(BF16)

    w2t = np.ascontiguousarray(
        w2.T.reshape(KT2, 128, H2).transpose(1, 0, 2).reshape(128, KT2 * H2)
    ).astype(BF16)

    in_maps = [
        {"x": np.ascontiguousarray(xpad[c * B_PER:(c + 1) * B_PER]),
         "w1t": w1t, "w2t": w2t}
        for c in range(N_CORES)
    ]
    res = run_bass_kernel_spmd(nc, in_maps, core_ids=list(range(N_CORES)))
    out = np.stack([res.results[c]["y"] for c in range(N_CORES)])  # (8, 20, 2*500)
    out = out.reshape(N_CORES, H2, B_PER, T).transpose(0, 2, 1, 3)
    return np.ascontiguousarray(
        out.reshape(N_CORES * B_PER, H2, T).astype(np.float32))


# revision 7
# speedup vs baseline: 1.1952x; 1.1952x over previous
"""Trainium2 Bass kernel for the SLAYER-style 2-layer spiking encoder.

Pipeline per core (2 batches per core, 8 cores, pure data-parallel over batch):
  fc1 (PE, bf16, k-streamed)  ->  alpha-psp scans (DVE tensor_tensor_scan)
  -> membrane scatter into step-major layout (GpSimd + ACT)
  -> layer-1 spike chain (DVE, 4 fused ops/step, time-chunked 10x50+30 warmup)
  -> fc2 (PE, strided read of spike store)  -> alpha-psp scans
  -> layer-2 spike chain (DVE, 4 ops/step, chunked 20x25+30 warmup)
  -> DMA out (host divides by cref to recover 0/1 spikes)

Key algebraic facts exploited:
  * alpha_psp is linear and commutes with the feature-contracting matmuls:
    we matmul raw binary spikes (exact in bf16) and filter the (T,1024) result.
  * alpha_psp = two cascaded one-pole recurrences -> two tensor_tensor_scan
    instructions per tile:  p[t] = d*p[t-1] + v[t];  r[t] = d*r[t-1] + p[t];
    u[t] = c*d*r[t-1].
  * spike_dyn state decays by e^-1 per step, so time chunks processed in
    parallel SIMD lanes from a zero state agree with the sequential result
    after a 30-step warmup (state error ~1e-11, decision margin >> that).
  * the spike store holds cref*s (bf16-exact since cref is snapped to bf16);
    1/cref is folded into the next layer's scan epilogue / host rescale.

Layer-1 membrane U and spike store S use a step-major layout
(col = step*80 + group*10 + chunk) so every chain op touches contiguous SBUF.
fc2 reads the spike store back in time order through a transposed strided AP.
"""

import numpy as np
import ml_dtypes

# ---------------------------------------------------------------- constants
B_TOT = 16
B_PER = 2
N_CORES = 8
T = 500
F_IN = 6300
F_PAD = 6400          # 50 k-tiles of 128
H1 = 1024             # fc1 output neurons -> 8 o-tiles of 128
H2 = 20               # fc2 output neurons
KT1 = F_PAD // 128    # 50
OT1 = H1 // 128       # 8
KT2 = H1 // 128       # 8

THETA = 10.0
D = float(np.float32(np.exp(-1.0)))        # decay per step
C = float(np.float32(np.e))                # alpha kernel scale (tau=1)
CD = C * D                                 # u = c*q = c*d*r[t-1]
# refractory scale, snapped to the nearest bf16 so device-side bf16 stores of
# cref*s are exact; the inverse is folded into downstream scales.
CREF = float(np.asarray(-2.0 * THETA * np.e, dtype=ml_dtypes.bfloat16).astype(np.float32))

WARM = 30             # spike-chain warmup steps (state decays e^-1/step)
# layer-1 chain: 10 chunks x 50 steps
NCH1, CHL1 = 10, 50
NSTEP1 = CHL1 + WARM  # 80
LAN1 = OT1 * NCH1     # 80 lanes per partition-row per step
# layer-2 chain: 20 chunks x 25 steps
NCH2, CHL2 = 20, 25
NSTEP2 = CHL2 + WARM  # 55

BF16 = ml_dtypes.bfloat16
_CACHE = {}


def _chunk_slices(i, nch, chl):
    """Active chunk range and in-chunk column for chain step i."""
    t0 = i - WARM                       # time of chunk 0 at this step
    j0 = 0 if t0 >= 0 else (-t0 + chl - 1) // chl
    c = t0 + j0 * chl                   # in [0, chl)
    return j0, c


def _build():
    import concourse.bass as bass
    import concourse.bacc as bacc
    import concourse.mybir as mybir
    import concourse.tile as tile

    f32 = mybir.dt.float32
    bf16 = mybir.dt.bfloat16
    MULT = mybir.AluOpType.mult
    ADD = mybir.AluOpType.add
    IS_GE = mybir.AluOpType.is_ge
    COPY = mybir.ActivationFunctionType.Copy

    nc = bacc.Bacc("TRN2", target_bir_lowering=False, debug=False,
                   num_devices=N_CORES)

    x_d = nc.dram_tensor("x", [B_PER, F_PAD, T], bf16, kind="ExternalInput").ap()
    w1t_d = nc.dram_tensor("w1t", [F_PAD, H1], bf16, kind="ExternalInput").ap()
    w2t_d = nc.dram_tensor("w2t", [128, KT2 * H2], bf16, kind="ExternalInput").ap()
    y_d = nc.dram_tensor("y", [H2, B_PER * T], f32, kind="ExternalOutput").ap()

    with tile.TileContext(nc) as tc:
        with (
            tc.tile_pool(name="xs", bufs=4) as xsp,
            tc.tile_pool(name="w1k", bufs=4) as w1kp,
            tc.tile_pool(name="wee", bufs=1) as wee,
            tc.tile_pool(name="ust", bufs=2) as ustp,
            tc.tile_pool(name="sst", bufs=2) as sstp,
            tc.tile_pool(name="scan", bufs=4) as scanp,
            tc.tile_pool(name="cst", bufs=1) as cstp,
            tc.tile_pool(name="state", bufs=6) as statep,
            tc.tile_pool(name="l2", bufs=1) as l2p,
            tc.tile_pool(name="ps", bufs=8, space="PSUM") as psp,
        ):
            # ---- constants
            dconst = cstp.tile([128, T], f32, tag="dconst")
            nc.gpsimd.memset(dconst[:], D)
            crefc = cstp.tile([128, LAN1], f32, tag="crefc")
            nc.gpsimd.memset(crefc[:], CREF)
            cref2 = cstp.tile([H2, B_PER * NCH2], f32, tag="cref2")
            nc.gpsimd.memset(cref2[:], CREF)

            w2sb = wee.tile([128, KT2 * H2], bf16, tag="w2sb")
            nc.sync.dma_start(w2sb[:], w2t_d[:])

            sst = []
            # ============== per-batch: fc1 + scans + scatter + spike chain
            for b in range(B_PER):
                u_b = ustp.tile([128, NSTEP1 * LAN1], f32, tag="ust",
                                name=f"u_{b}")
                s_b = sstp.tile([128, NSTEP1 * LAN1], bf16, tag="sst",
                                name=f"s_{b}")
                sst.append(s_b)
                u4 = u_b[:].rearrange("p (i g j) -> p i g j", i=NSTEP1, g=OT1)
                s4 = s_b[:].rearrange("p (i g j) -> p i g j", i=NSTEP1, g=OT1)

                # fc1: v1[o, t] += w1t[k, o] * x[k, t]   (x streamed per k-tile)
                xsrc = x_d[b].rearrange("(k p) t -> p k t", p=128)
                v1 = [psp.tile([128, T], f32, tag="ps", name=f"v1_{b}_{ot}")
                      for ot in range(OT1)]
                for k in range(KT1):
                    xk = xsp.tile([128, T], bf16, tag="xs", name=f"x_{b}_{k}")
                    nc.sync.dma_start(xk[:], xsrc[:, k, :])
                    w1k = w1kp.tile([128, H1], bf16, tag="w1k", name=f"w1_{b}_{k}")
                    nc.sync.dma_start(w1k[:], w1t_d[k * 128:(k + 1) * 128, :])
                    for ot in range(OT1):
                        nc.tensor.matmul(
                            v1[ot][:],
                            w1k[:, ot * 128:(ot + 1) * 128],
                            xk[:],
                            start=(k == 0), stop=(k == KT1 - 1),
                        )

                # alpha-psp scans (DVE) + membrane scatter into step-major U:
                #   U[t] = cd*r[t-1] - theta   at (i, g, j) with t = j*CHL1+i-WARM
                nc.gpsimd.memset(u4[:, WARM, :, 0], -THETA)   # t=0 columns
                for ot in range(OT1):
                    p_t = scanp.tile([128, T], f32, tag="scan", name=f"p_{b}_{ot}")
                    r_t = scanp.tile([128, T], f32, tag="scan", name=f"r_{b}_{ot}")
                    nc.vector.tensor_tensor_scan(
                        p_t[:], dconst[:], v1[ot][:], 0.0, op0=MULT, op1=ADD)
                    nc.vector.tensor_tensor_scan(
                        r_t[:], dconst[:], p_t[:], 0.0, op0=MULT, op1=ADD)
                    # output phase (i in [WARM, NSTEP1)): chunk j, t = j*CHL1+c
                    nc.gpsimd.tensor_scalar(
                        u4[:, WARM + 1:, ot, 0], r_t[:, 0:CHL1 - 1],
                        CD, -THETA, op0=MULT, op1=ADD)
                    for j in range(1, NCH1):
                        nc.gpsimd.tensor_scalar(
                            u4[:, WARM:, ot, j],
                            r_t[:, j * CHL1 - 1:(j + 1) * CHL1 - 1],
                            CD, -THETA, op0=MULT, op1=ADD)
                    # warmup copies (i in [0, WARM), chunk j>=1 reads its own
                    # pre-chunk history): t = j*CHL1 + i - WARM
                    for j in range(1, NCH1):
                        nc.scalar.activation(
                            u4[:, 0:WARM, ot, j],
                            r_t[:, j * CHL1 - WARM - 1:j * CHL1 - 1],
                            COPY, bias=-THETA, scale=CD)

                # layer-1 spike chain:
                #   Z = d*Z + P ; M = d*Z + U_i ; S_i = (M>=0)*cref ; P = d*P + S_i
                zt = statep.tile([128, LAN1], f32, tag="state", name=f"z_{b}")
                pt = statep.tile([128, LAN1], f32, tag="state", name=f"pp_{b}")
                mt = statep.tile([128, LAN1], f32, tag="state", name=f"m_{b}")
                nc.gpsimd.memset(zt[:], 0.0)
                nc.gpsimd.memset(pt[:], 0.0)
                z3 = zt[:].rearrange("p (g j) -> p g j", g=OT1)
                p3 = pt[:].rearrange("p (g j) -> p g j", g=OT1)
                m3 = mt[:].rearrange("p (g j) -> p g j", g=OT1)
                c3 = crefc[:].rearrange("p (g j) -> p g j", g=OT1)
                for i in range(NSTEP1):
                    j0, ci = _chunk_slices(i, NCH1, CHL1)
                    if j0 == 0:  # contiguous fast path
                        zs, ps_, ms = zt[:], pt[:], mt[:]
                        cs = crefc[:]
                        us = u_b[:, i * LAN1:(i + 1) * LAN1]
                        ss = s_b[:, i * LAN1:(i + 1) * LAN1]
                    else:
                        zs, ps_, ms = z3[:, :, j0:], p3[:, :, j0:], m3[:, :, j0:]
                        cs = c3[:, :, j0:]
                        us = u4[:, i, :, 0:NCH1 - j0]
                        ss = s4[:, i, :, 0:NCH1 - j0]
                    nc.vector.scalar_tensor_tensor(zs, zs, D, ps_, op0=MULT, op1=ADD)
                    nc.vector.scalar_tensor_tensor(ms, zs, D, us, op0=MULT, op1=ADD)
                    nc.vector.scalar_tensor_tensor(ss, ms, 0.0, cs, op0=IS_GE, op1=MULT)
                    nc.vector.scalar_tensor_tensor(ps_, ps_, D, ss, op0=MULT, op1=ADD)

            # ============== layer 2 (PSUM banks free again)
            u2 = l2p.tile([H2, B_PER * T], f32, tag="u2")
            s2 = l2p.tile([H2, B_PER * T], f32, tag="s2")
            for b in range(B_PER):
                v2 = psp.tile([H2, T], f32, tag="ps", name=f"v2_{b}")
                sr = sst[b][:].rearrange("p (i g j) -> p i g j", i=NSTEP1, g=OT1)
                for kt in range(KT2):
                    # read spikes back in time order: (j outer, c inner)
                    rhs = sr[:, WARM:, kt, :].transpose([0, 2, 1])
                    nc.tensor.matmul(
                        v2[:],
                        w2sb[:, kt * H2:(kt + 1) * H2],
                        rhs,
                        start=(kt == 0), stop=(kt == KT2 - 1),
                    )
                p2t = scanp.tile([H2, T], f32, tag="scan", name=f"p2_{b}")
                r2t = scanp.tile([H2, T], f32, tag="scan", name=f"r2_{b}")
                nc.vector.tensor_tensor_scan(
                    p2t[:], dconst[0:H2, :], v2[:], 0.0, op0=MULT, op1=ADD)
                nc.vector.tensor_tensor_scan(
                    r2t[:], dconst[0:H2, :], p2t[:], 0.0, op0=MULT, op1=ADD)
                c0 = b * T
                nc.gpsimd.memset(u2[:, c0:c0 + 1], -THETA)
                nc.scalar.activation(
                    u2[:, c0 + 1:c0 + T], r2t[:, 0:T - 1], COPY,
                    bias=-THETA, scale=CD / CREF)

            # layer-2 spike chain; the store keeps cref*s (host divides by cref)
            z2 = l2p.tile([H2, B_PER * NCH2], f32, tag="z2")
            pp2 = l2p.tile([H2, B_PER * NCH2], f32, tag="pp2")
            m2 = l2p.tile([H2, B_PER * NCH2], f32, tag="m2")
            nc.gpsimd.memset(z2[:], 0.0)
            nc.gpsimd.memset(pp2[:], 0.0)
            z2v = z2[:].rearrange("p (b j) -> p b j", b=B_PER)
            pp2v = pp2[:].rearrange("p (b j) -> p b j", b=B_PER)
            m2v = m2[:].rearrange("p (b j) -> p b j", b=B_PER)
            c2v = cref2[:].rearrange("p (b j) -> p b j", b=B_PER)
            u24 = u2[:].rearrange("p (b j c) -> p b j c", b=B_PER, j=NCH2)
            s24 = s2[:].rearrange("p (b j c) -> p b j c", b=B_PER, j=NCH2)
            for i in range(NSTEP2):
                j0, ci = _chunk_slices(i, NCH2, CHL2)
                zs, ps_ = z2v[:, :, j0:], pp2v[:, :, j0:]
                ms, cs = m2v[:, :, j0:], c2v[:, :, j0:]
                us, ss = u24[:, :, 0:NCH2 - j0, ci], s24[:, :, 0:NCH2 - j0, ci]
                nc.vector.scalar_tensor_tensor(zs, zs, D, ps_, op0=MULT, op1=ADD)
                nc.vector.scalar_tensor_tensor(ms, zs, D, us, op0=MULT, op1=ADD)
                nc.vector.scalar_tensor_tensor(ss, ms, 0.0, cs, op0=IS_GE, op1=MULT)
                nc.vector.scalar_tensor_tensor(ps_, ps_, D, ss, op0=MULT, op1=ADD)

            nc.sync.dma_start(y_d[:], s2[:])

    nc.compile()
    return nc


def _get_nc():
    if "nc" not in _CACHE:
        _CACHE["nc"] = _build()
    return _CACHE["nc"]


def _prep_inputs(downsampled, w1, w2):
    x = np.ascontiguousarray(downsampled.reshape(B_TOT, F_IN, T))
    xpad = np.zeros((B_TOT, F_PAD, T), dtype=BF16)
    xpad[:, :F_IN] = x.astype(BF16)
    w1t = np.zeros((F_PAD, H1), dtype=BF16)
    w1t[:F_IN] = np.ascontiguousarray(w1.T).astype(BF16)
    w2t = np.ascontiguousarray(
        w2.T.reshape(KT2, 128, H2).transpose(1, 0, 2).reshape(128, KT2 * H2)
    ).astype(BF16)
    return [
        {"x": np.ascontiguousarray(xpad[c * B_PER:(c + 1) * B_PER]),
         "w1t": w1t, "w2t": w2t}
        for c in range(N_CORES)
    ]


def kernel(downsampled: np.ndarray, w1: np.ndarray, w2: np.ndarray) -> np.ndarray:
    from concourse.bass_utils import run_bass_kernel_spmd

    nc = _get_nc()
    in_maps = _prep_inputs(downsampled, w1, w2)
    res = run_bass_kernel_spmd(nc, in_maps, core_ids=list(range(N_CORES)))
    out = np.stack([res.results[c]["y"] for c in range(N_CORES)])  # (8, 20, 2T)
    out = out.reshape(N_CORES, H2, B_PER, T).transpose(0, 2, 1, 3)
    out = out.reshape(B_TOT, H2, T) / np.float32(CREF)   # cref*s -> s (exact)
    return np.ascontiguousarray(out.astype(np.float32))


# revision 8
# speedup vs baseline: 1.4745x; 1.2337x over previous
"""Trainium2 Bass kernel for the SLAYER-style 2-layer spiking encoder.

Pipeline per core (2 batches per core, 8 cores, pure data-parallel over batch):
  fc1 (PE, fp8-e4m3 DoubleRow, k-streamed)  ->  alpha-psp scans (DVE
  tensor_tensor_scan)  ->  membrane scatter into step-major layout
  (GpSimd + DVE + ACT)  ->  layer-1 spike chain (DVE, 4 fused
  scalar_tensor_tensor ops/step, both batches in one 160-lane chain,
  time-chunked 10x50 + 30-step warmup)  ->  fc2 (PE, strided read of the
  spike store)  ->  alpha-psp scans  ->  layer-2 spike chain (20 chunks x 25)
  ->  DMA out (host divides by cref to recover 0/1 spikes).

Key algebraic facts exploited:
  * alpha_psp is linear and commutes with the feature-contracting matmuls:
    matmul the raw binary spikes (exactly representable in fp8/bf16), filter
    the (T,1024) result instead of (T,6300).
  * alpha_psp = two cascaded one-pole recurrences -> two tensor_tensor_scan
    instructions per tile:  p[t] = d*p[t-1] + v[t];  r[t] = d*r[t-1] + p[t];
    u[t] = c*d*r[t-1] - theta.
  * spike_dyn state decays by e^-1 per step, so time chunks processed in
    parallel SIMD lanes from zero state match the sequential result after a
    30-step warmup (residual ~1e-11, decision margins are vastly larger).
  * the spike store holds cref*s with cref snapped to bf16 (so the store is
    exact); 1/cref is folded into the next layer's scan scale / host rescale.

Layer-1 membrane U and spike store S are step-major
(col = step*160 + batch*80 + group*10 + chunk) so chain ops touch contiguous
SBUF; fc2 reads spikes back in time order through a transposed strided AP.
"""

import numpy as np
import ml_dtypes

# ---------------------------------------------------------------- constants
B_TOT = 16
B_PER = 2
N_CORES = 8
T = 500
F_IN = 6300
F_PAD = 6400
H1 = 1024
H2 = 20
KP1 = F_PAD // 256    # 25 fp8 DoubleRow k-pair tiles
OT1 = H1 // 128       # 8
KT2 = H1 // 128       # 8

THETA = 10.0
D = float(np.float32(np.exp(-1.0)))
C = float(np.float32(np.e))
CD = C * D
CREF = float(np.asarray(-2.0 * THETA * np.e, dtype=ml_dtypes.bfloat16).astype(np.float32))

WARM = 30
NCH1, CHL1 = 10, 50
NSTEP1 = CHL1 + WARM          # 80
LAN1 = B_PER * OT1 * NCH1     # 160 chain lanes per partition-row
NCH2, CHL2 = 20, 25
NSTEP2 = CHL2 + WARM          # 55

BF16 = ml_dtypes.bfloat16
E4M3 = ml_dtypes.float8_e4m3
_CACHE = {}


def _chunk_slices(i, chl):
    """(first active chunk j0, in-chunk column c) for chain step i."""
    t0 = i - WARM
    j0 = 0 if t0 >= 0 else (-t0 + chl - 1) // chl
    return j0, t0 + j0 * chl


def _build():
    import concourse.bass as bass
    import concourse.bacc as bacc
    import concourse.mybir as mybir
    import concourse.tile as tile

    f32 = mybir.dt.float32
    bf16 = mybir.dt.bfloat16
    fp8 = mybir.dt.float8e4
    MULT = mybir.AluOpType.mult
    ADD = mybir.AluOpType.add
    IS_GE = mybir.AluOpType.is_ge
    COPY = mybir.ActivationFunctionType.Copy
    DROW = mybir.MatmulPerfMode.DoubleRow

    nc = bacc.Bacc("TRN2", target_bir_lowering=False, debug=False,
                   num_devices=N_CORES)

    x_d = nc.dram_tensor("x", [B_PER, F_PAD, T], fp8, kind="ExternalInput").ap()
    w1t_d = nc.dram_tensor("w1t", [F_PAD, H1], fp8, kind="ExternalInput").ap()
    w2t_d = nc.dram_tensor("w2t", [128, KT2 * H2], bf16, kind="ExternalInput").ap()
    y_d = nc.dram_tensor("y", [H2, B_PER * T], f32, kind="ExternalOutput").ap()

    with tile.TileContext(nc) as tc:
        with (
            tc.tile_pool(name="xs", bufs=4) as xsp,
            tc.tile_pool(name="w1k", bufs=4) as w1kp,
            tc.tile_pool(name="wee", bufs=1) as wee,
            tc.tile_pool(name="ust", bufs=1) as ustp,
            tc.tile_pool(name="sst", bufs=1) as sstp,
            tc.tile_pool(name="scan", bufs=4) as scanp,
            tc.tile_pool(name="cst", bufs=1) as cstp,
            tc.tile_pool(name="state", bufs=3) as statep,
            tc.tile_pool(name="l2", bufs=1) as l2p,
            tc.tile_pool(name="ps", bufs=8, space="PSUM") as psp,
        ):
            # ---- constants
            dconst = cstp.tile([128, T], f32, tag="dconst")
            nc.gpsimd.memset(dconst[:], D)
            crefc = cstp.tile([128, LAN1], bf16, tag="crefc")
            nc.gpsimd.memset(crefc[:], CREF)
            cref2 = cstp.tile([H2, B_PER * NCH2], f32, tag="cref2")
            nc.gpsimd.memset(cref2[:], CREF)

            w2sb = wee.tile([128, KT2 * H2], bf16, tag="w2sb")
            nc.sync.dma_start(w2sb[:], w2t_d[:])

            # step-major membrane / spike stores (both batches interleaved)
            u_st = ustp.tile([128, NSTEP1 * LAN1], f32, tag="ust")
            s_st = sstp.tile([128, NSTEP1 * LAN1], bf16, tag="sst")
            u5 = u_st[:].rearrange("p (i b g j) -> p i b g j",
                                   i=NSTEP1, b=B_PER, g=OT1)
            s5 = s_st[:].rearrange("p (i b g j) -> p i b g j",
                                   i=NSTEP1, b=B_PER, g=OT1)
            nc.gpsimd.memset(u5[:, WARM, :, :, 0], -THETA)   # t = 0 columns

            # ============== per-batch fc1 + scans + scatter
            for b in range(B_PER):
                # fc1: v1[o, t] += w1t[k, o] * x[k, t], fp8 DoubleRow (K=256/mm)
                xsrc = x_d[b].rearrange("(kp s p) t -> p kp s t", s=2, p=128)
                wsrc = w1t_d.rearrange("(kp s p) o -> p kp s o", s=2, p=128)
                v1 = [psp.tile([128, T], f32, tag="ps", name=f"v1_{b}_{ot}")
                      for ot in range(OT1)]
                for kp in range(KP1):
                    xk = xsp.tile([128, 2 * T], fp8, tag="xs", name=f"x_{b}_{kp}")
                    nc.sync.dma_start(
                        xk[:].rearrange("p (s t) -> p s t", s=2), xsrc[:, kp])
                    w1k = w1kp.tile([128, 2 * H1], fp8, tag="w1k",
                                    name=f"w1_{b}_{kp}")
                    nc.sync.dma_start(
                        w1k[:].rearrange("p (s o) -> p s o", s=2), wsrc[:, kp])
                    w3 = w1k[:].rearrange("p (s o) -> p s o", s=2)
                    x3 = xk[:].rearrange("p (s t) -> p s t", s=2)
                    for ot in range(OT1):
                        nc.tensor.matmul(
                            v1[ot][:],
                            w3[:, :, ot * 128:(ot + 1) * 128],
                            x3,
                            start=(kp == 0), stop=(kp == KP1 - 1),
                            perf_mode=DROW,
                        )

                # alpha-psp scans (DVE) + membrane scatter into step-major U:
                #   U[i, b, g, j] = cd*r[t-1] - theta,  t = j*CHL1 + i - WARM
                for ot in range(OT1):
                    p_t = scanp.tile([128, T], f32, tag="scan", name=f"p_{b}_{ot}")
                    r_t = scanp.tile([128, T], f32, tag="scan", name=f"r_{b}_{ot}")
                    nc.vector.tensor_tensor_scan(
                        p_t[:], dconst[:], v1[ot][:], 0.0, op0=MULT, op1=ADD)
                    nc.vector.tensor_tensor_scan(
                        r_t[:], dconst[:], p_t[:], 0.0, op0=MULT, op1=ADD)
                    # output phase (steps >= WARM): 10 strided copies
                    eng = (nc.gpsimd if ot < 3 else
                           (nc.vector if ot < 6 else nc.gpsimd))
                    eng.tensor_scalar(
                        u5[:, WARM + 1:, b, ot, 0], r_t[:, 0:CHL1 - 1],
                        CD, -THETA, op0=MULT, op1=ADD)
                    for j in range(1, NCH1):
                        eng.tensor_scalar(
                            u5[:, WARM:, b, ot, j],
                            r_t[:, j * CHL1 - 1:(j + 1) * CHL1 - 1],
                            CD, -THETA, op0=MULT, op1=ADD)
                    # warmup copies (steps < WARM, chunk j reads its history)
                    for j in range(1, NCH1):
                        nc.scalar.activation(
                            u5[:, 0:WARM, b, ot, j],
                            r_t[:, j * CHL1 - WARM - 1:j * CHL1 - 1],
                            COPY, bias=-THETA, scale=CD)

            # ============== layer-1 spike chain (both batches, 160 lanes)
            #   Z = d*Z + P ; M = d*Z + U_i ; S_i = (M>=0)*cref ; P = d*P + S_i
            zt = statep.tile([128, LAN1], bf16, tag="state", name="z1")
            pt = statep.tile([128, LAN1], bf16, tag="state", name="pp1")
            mt = statep.tile([128, LAN1], bf16, tag="state", name="m1")
            nc.gpsimd.memset(zt[:], 0.0)
            nc.gpsimd.memset(pt[:], 0.0)
            z5 = zt[:].rearrange("p (b g j) -> p b g j", b=B_PER, g=OT1)
            p5 = pt[:].rearrange("p (b g j) -> p b g j", b=B_PER, g=OT1)
            m5 = mt[:].rearrange("p (b g j) -> p b g j", b=B_PER, g=OT1)
            c5 = crefc[:].rearrange("p (b g j) -> p b g j", b=B_PER, g=OT1)
            for i in range(NSTEP1):
                j0, ci = _chunk_slices(i, CHL1)
                if j0 == 0:  # contiguous fast path
                    zs, ps_, ms, cs = zt[:], pt[:], mt[:], crefc[:]
                    us = u_st[:, i * LAN1:(i + 1) * LAN1]
                    ss = s_st[:, i * LAN1:(i + 1) * LAN1]
                else:
                    zs, ps_ = z5[:, :, :, j0:], p5[:, :, :, j0:]
                    ms, cs = m5[:, :, :, j0:], c5[:, :, :, j0:]
                    us = u5[:, i, :, :, 0:NCH1 - j0]
                    ss = s5[:, i, :, :, 0:NCH1 - j0]
                nc.vector.scalar_tensor_tensor(zs, zs, D, ps_, op0=MULT, op1=ADD)
                nc.vector.scalar_tensor_tensor(ms, zs, D, us, op0=MULT, op1=ADD)
                nc.vector.scalar_tensor_tensor(ss, ms, 0.0, cs, op0=IS_GE, op1=MULT)
                nc.vector.scalar_tensor_tensor(ps_, ps_, D, ss, op0=MULT, op1=ADD)

            # ============== layer 2
            u2 = l2p.tile([H2, B_PER * T], f32, tag="u2")
            s2 = l2p.tile([H2, B_PER * T], f32, tag="s2")
            for b in range(B_PER):
                v2 = psp.tile([H2, T], f32, tag="ps", name=f"v2_{b}")
                for kt in range(KT2):
                    rhs = s5[:, WARM:, b, kt, :].transpose([0, 2, 1])
                    nc.tensor.matmul(
                        v2[:],
                        w2sb[:, kt * H2:(kt + 1) * H2],
                        rhs,
                        start=(kt == 0), stop=(kt == KT2 - 1),
                    )
                p2t = scanp.tile([H2, T], f32, tag="scan", name=f"p2_{b}")
                r2t = scanp.tile([H2, T], f32, tag="scan", name=f"r2_{b}")
                nc.vector.tensor_tensor_scan(
                    p2t[:], dconst[0:H2, :], v2[:], 0.0, op0=MULT, op1=ADD)
                nc.vector.tensor_tensor_scan(
                    r2t[:], dconst[0:H2, :], p2t[:], 0.0, op0=MULT, op1=ADD)
                c0 = b * T
                nc.gpsimd.memset(u2[:, c0:c0 + 1], -THETA)
                nc.scalar.activation(
                    u2[:, c0 + 1:c0 + T], r2t[:, 0:T - 1], COPY,
                    bias=-THETA, scale=CD / CREF)

            # layer-2 spike chain; store keeps cref*s (host divides by cref)
            z2 = l2p.tile([H2, B_PER * NCH2], f32, tag="z2")
            pp2 = l2p.tile([H2, B_PER * NCH2], f32, tag="pp2")
            m2 = l2p.tile([H2, B_PER * NCH2], f32, tag="m2")
            nc.gpsimd.memset(z2[:], 0.0)
            nc.gpsimd.memset(pp2[:], 0.0)
            z2v = z2[:].rearrange("p (b j) -> p b j", b=B_PER)
            pp2v = pp2[:].rearrange("p (b j) -> p b j", b=B_PER)
            m2v = m2[:].rearrange("p (b j) -> p b j", b=B_PER)
            c2v = cref2[:].rearrange("p (b j) -> p b j", b=B_PER)
            u24 = u2[:].rearrange("p (b j c) -> p b j c", b=B_PER, j=NCH2)
            s24 = s2[:].rearrange("p (b j c) -> p b j c", b=B_PER, j=NCH2)
            for i in range(NSTEP2):
                j0, ci = _chunk_slices(i, CHL2)
                zs, ps_ = z2v[:, :, j0:], pp2v[:, :, j0:]
                ms, cs = m2v[:, :, j0:], c2v[:, :, j0:]
                us, ss = u24[:, :, 0:NCH2 - j0, ci], s24[:, :, 0:NCH2 - j0, ci]
                nc.vector.scalar_tensor_tensor(zs, zs, D, ps_, op0=MULT, op1=ADD)
                nc.vector.scalar_tensor_tensor(ms, zs, D, us, op0=MULT, op1=ADD)
                nc.vector.scalar_tensor_tensor(ss, ms, 0.0, cs, op0=IS_GE, op1=MULT)
                nc.vector.scalar_tensor_tensor(ps_, ps_, D, ss, op0=MULT, op1=ADD)

            nc.sync.dma_start(y_d[:], s2[:])

    nc.compile()
    return nc


def _get_nc():
    if "nc" not in _CACHE:
        _CACHE["nc"] = _build()
    return _CACHE["nc"]


def _prep_inputs(downsampled, w1, w2):
    x = np.ascontiguousarray(downsampled.reshape(B_TOT, F_IN, T))
    xpad = np.zeros((B_TOT, F_PAD, T), dtype=E4M3)
    xpad[:, :F_IN] = x.astype(E4M3)          # binary spikes: exact in e4m3
    w1t = np.zeros((F_PAD, H1), dtype=E4M3)
    w1t[:F_IN] = np.ascontiguousarray(w1.T).astype(E4M3)
    w2t = np.ascontiguousarray(
        w2.T.reshape(KT2, 128, H2).transpose(1, 0, 2).reshape(128, KT2 * H2)
    ).astype(BF16)
    return [
        {"x": np.ascontiguousarray(xpad[c * B_PER:(c + 1) * B_PER]),
         "w1t": w1t, "w2t": w2t}
        for c in range(N_CORES)
    ]


def kernel(downsampled: np.ndarray, w1: np.ndarray, w2: np.ndarray) -> np.ndarray:
    from concourse.bass_utils import run_bass_kernel_spmd

    nc = _get_nc()
    in_maps = _prep_inputs(downsampled, w1, w2)
    res = run_bass_kernel_spmd(nc, in_maps, core_ids=list(range(N_CORES)))
    out = np.stack([res.results[c]["y"] for c in range(N_CORES)])  # (8, 20, 2T)
    out = out.reshape(N_CORES, H2, B_PER, T).transpose(0, 2, 1, 3)
    out = out.reshape(B_TOT, H2, T) / np.float32(CREF)   # cref*s -> s (exact)
    return np.ascontiguousarray(out.astype(np.float32))


# revision 10
# speedup vs baseline: 1.5829x; 1.0735x over previous
"""Trainium2 Bass kernel for the SLAYER-style 2-layer spiking encoder.

Pipeline per core (2 batches per core, 8 cores, pure data-parallel over batch):
  fc1 (PE, fp8-e4m3 DoubleRow, k-streamed)  ->  alpha-psp scans
  (batch 0: ACT psum-copy + GpSimd scans, hidden under batch 1's fc1;
   batch 1: DVE scans)  ->  membrane epilogue (ACT)  ->  layer-1 spike chain
  (DVE, 4 fused scalar_tensor_tensor ops/step, both batches in one 320-lane
  chain, 20 time chunks x 25 steps + 20-step warmup)  ->  fc2 (PE, strided
  read of the step-major spike store)  ->  alpha-psp scans  ->  layer-2
  spike chain (25 chunks x 20)  ->  DMA out (host divides by cref).

Key algebraic facts exploited:
  * alpha_psp is linear and commutes with the feature-contracting matmuls:
    matmul the raw binary spikes (exactly representable in fp8/bf16), filter
    the (T,1024) result instead of (T,6300).
  * alpha_psp = two cascaded one-pole recurrences -> two tensor_tensor_scan
    instructions per tile:  p[t] = d*p[t-1] + v[t];  r[t] = d*r[t-1] + p[t];
    membrane drive  u[t] = c*d*r[t-1] - theta.
  * spike_dyn state decays by e^-1 per step, so time chunks processed in
    parallel SIMD lanes from zero state match the sequential result after a
    20-step warmup (residual ~2e-7 vs. empirical decision margins ~1e-3 for
    layer 1 and ~9.0 for layer 2).
  * the spike store holds cref*s with cref snapped to bf16 (exact store);
    1/cref is folded into the next layer's scan scale / host rescale.

The membrane U is t-major; the chain reads it through a 4-D strided AP.
The spike store S is step-major (contiguous per chain step); fc2 reads it
back in time order through a transposed strided AP.
"""

import numpy as np
import ml_dtypes

# ---------------------------------------------------------------- constants
B_TOT = 16
B_PER = 2
N_CORES = 8
T = 500
F_IN = 6300
F_PAD = 6400
H1 = 1024
H2 = 20
KP1 = F_PAD // 256    # 25 fp8 DoubleRow k-pair tiles
OT1 = H1 // 128       # 8
KT2 = H1 // 128       # 8

THETA = 10.0
D = float(np.float32(np.exp(-1.0)))
C = float(np.float32(np.e))
CD = C * D
CREF = float(np.asarray(-2.0 * THETA * np.e, dtype=ml_dtypes.bfloat16).astype(np.float32))

WARM = 20
NCH1, CHL1 = 20, 25
NSTEP1 = CHL1 + WARM          # 45
LAN1 = B_PER * OT1 * NCH1     # 320 chain lanes per partition-row
NCH2, CHL2 = 25, 20
NSTEP2 = CHL2 + WARM          # 40

BF16 = ml_dtypes.bfloat16
E4M3 = ml_dtypes.float8_e4m3
_CACHE = {}


def _chunk_slices(i, chl):
    """(first active chunk j0, in-chunk column c) for chain step i."""
    t0 = i - WARM
    j0 = 0 if t0 >= 0 else (-t0 + chl - 1) // chl
    return j0, t0 + j0 * chl


def _build():
    import concourse.bass as bass
    import concourse.bacc as bacc
    import concourse.mybir as mybir
    import concourse.tile as tile

    f32 = mybir.dt.float32
    bf16 = mybir.dt.bfloat16
    fp8 = mybir.dt.float8e4
    MULT = mybir.AluOpType.mult
    ADD = mybir.AluOpType.add
    IS_GE = mybir.AluOpType.is_ge
    COPY = mybir.ActivationFunctionType.Copy
    DROW = mybir.MatmulPerfMode.DoubleRow

    nc = bacc.Bacc("TRN2", target_bir_lowering=False, debug=False,
                   num_devices=N_CORES)

    x_d = nc.dram_tensor("x", [B_PER, F_PAD, T], fp8, kind="ExternalInput").ap()
    w1t_d = nc.dram_tensor("w1t", [F_PAD, H1], fp8, kind="ExternalInput").ap()
    w2t_d = nc.dram_tensor("w2t", [128, KT2 * H2], bf16, kind="ExternalInput").ap()
    y_d = nc.dram_tensor("y", [H2, B_PER * T], f32, kind="ExternalOutput").ap()

    with tile.TileContext(nc) as tc:
        with (
            tc.tile_pool(name="xs", bufs=4) as xsp,
            tc.tile_pool(name="w1k", bufs=4) as w1kp,
            tc.tile_pool(name="wee", bufs=1) as wee,
            tc.tile_pool(name="ust", bufs=1) as ustp,
            tc.tile_pool(name="sst", bufs=1) as sstp,
            tc.tile_pool(name="scan", bufs=6) as scanp,
            tc.tile_pool(name="cst", bufs=1) as cstp,
            tc.tile_pool(name="state", bufs=3) as statep,
            tc.tile_pool(name="l2", bufs=1) as l2p,
            tc.tile_pool(name="ps", bufs=8, space="PSUM") as psp,
        ):
            # ---- constants
            dconst = cstp.tile([128, T], f32, tag="dconst")
            nc.gpsimd.memset(dconst[:], D)
            crefc = cstp.tile([128, LAN1], bf16, tag="crefc")
            nc.gpsimd.memset(crefc[:], CREF)
            cref2 = cstp.tile([H2, B_PER * NCH2], f32, tag="cref2")
            nc.gpsimd.memset(cref2[:], CREF)

            w2sb = wee.tile([128, KT2 * H2], bf16, tag="w2sb")
            nc.sync.dma_start(w2sb[:], w2t_d[:])

            # t-major membrane store U[b, g, t] = c*d*r[t-1] - theta
            u_tm = ustp.tile([128, B_PER * OT1 * T], f32, tag="ust")
            utm4 = u_tm[:].rearrange("p (b g t) -> p b g t", b=B_PER, g=OT1)
            # same columns viewed as chunks: t = j*CHL1 + c
            uj5 = u_tm[:].rearrange("p (b g j c) -> p b g j c",
                                    b=B_PER, g=OT1, j=NCH1)
            nc.gpsimd.memset(utm4[:, :, :, 0], -THETA)   # t = 0
            # step-major spike store (contiguous per chain step)
            s_st = sstp.tile([128, NSTEP1 * LAN1], bf16, tag="sst")
            s5 = s_st[:].rearrange("p (i b g j) -> p i b g j",
                                   i=NSTEP1, b=B_PER, g=OT1)

            # ============== per-batch fc1 + scans + membrane epilogue
            for b in range(B_PER):
                # fc1: v1[o, t] += w1t[k, o] * x[k, t], fp8 DoubleRow (K=256/mm)
                xsrc = x_d[b].rearrange("(kp s p) t -> p kp s t", s=2, p=128)
                wsrc = w1t_d.rearrange("(kp s p) o -> p kp s o", s=2, p=128)
                v1 = [psp.tile([128, T], f32, tag="ps", name=f"v1_{b}_{ot}")
                      for ot in range(OT1)]
                for kp in range(KP1):
                    xk = xsp.tile([128, 2 * T], fp8, tag="xs", name=f"x_{b}_{kp}")
                    nc.sync.dma_start(
                        xk[:].rearrange("p (s t) -> p s t", s=2), xsrc[:, kp])
                    w1k = w1kp.tile([128, 2 * H1], fp8, tag="w1k",
                                    name=f"w1_{b}_{kp}")
                    nc.sync.dma_start(
                        w1k[:].rearrange("p (s o) -> p s o", s=2), wsrc[:, kp])
                    w3 = w1k[:].rearrange("p (s o) -> p s o", s=2)
                    x3 = xk[:].rearrange("p (s t) -> p s t", s=2)
                    for ot in range(OT1):
                        nc.tensor.matmul(
                            v1[ot][:],
                            w3[:, :, ot * 128:(ot + 1) * 128],
                            x3,
                            start=(kp == 0), stop=(kp == KP1 - 1),
                            perf_mode=DROW,
                        )

                # alpha-psp scans (DVE) + membrane epilogue (ACT)
                for ot in range(OT1):
                    p_t = scanp.tile([128, T], f32, tag="scan", name=f"p_{b}_{ot}")
                    r_t = scanp.tile([128, T], f32, tag="scan", name=f"r_{b}_{ot}")
                    nc.vector.tensor_tensor_scan(
                        p_t[:], dconst[:], v1[ot][:], 0.0, op0=MULT, op1=ADD)
                    nc.vector.tensor_tensor_scan(
                        r_t[:], dconst[:], p_t[:], 0.0, op0=MULT, op1=ADD)
                    nc.scalar.activation(
                        utm4[:, b, ot, 1:T], r_t[:, 0:T - 1], COPY,
                        bias=-THETA, scale=CD)

            # ============== layer-1 spike chain (both batches, 320 lanes)
            #   Z = d*Z + P ; M = d*Z + U_i ; S_i = (M>=0)*cref ; P = d*P + S_i
            zt = statep.tile([128, LAN1], bf16, tag="state", name="z1")
            pt = statep.tile([128, LAN1], bf16, tag="state", name="pp1")
            mt = statep.tile([128, LAN1], bf16, tag="state", name="m1")
            nc.gpsimd.memset(zt[:], 0.0)
            nc.gpsimd.memset(pt[:], 0.0)
            z5 = zt[:].rearrange("p (b g j) -> p b g j", b=B_PER, g=OT1)
            p5 = pt[:].rearrange("p (b g j) -> p b g j", b=B_PER, g=OT1)
            m5 = mt[:].rearrange("p (b g j) -> p b g j", b=B_PER, g=OT1)
            c5 = crefc[:].rearrange("p (b g j) -> p b g j", b=B_PER, g=OT1)
            for i in range(NSTEP1):
                j0, ci = _chunk_slices(i, CHL1)
                us = uj5[:, :, :, j0:, ci]
                if j0 == 0:  # contiguous fast path for state/spike slabs
                    zs, ps_, ms, cs = zt[:], pt[:], mt[:], crefc[:]
                    ss = s_st[:, i * LAN1:(i + 1) * LAN1]
                else:
                    zs, ps_ = z5[:, :, :, j0:], p5[:, :, :, j0:]
                    ms, cs = m5[:, :, :, j0:], c5[:, :, :, j0:]
                    ss = s5[:, i, :, :, j0:]
                nc.vector.scalar_tensor_tensor(zs, zs, D, ps_, op0=MULT, op1=ADD)
                nc.vector.scalar_tensor_tensor(ms, zs, D, us, op0=MULT, op1=ADD)
                nc.vector.scalar_tensor_tensor(ss, ms, 0.0, cs, op0=IS_GE, op1=MULT)
                nc.vector.scalar_tensor_tensor(ps_, ps_, D, ss, op0=MULT, op1=ADD)

            # ============== layer 2
            u2 = l2p.tile([H2, B_PER * T], f32, tag="u2")
            s2 = l2p.tile([H2, B_PER * T], f32, tag="s2")
            for b in range(B_PER):
                v2 = psp.tile([H2, T], f32, tag="ps", name=f"v2_{b}")
                for kt in range(KT2):
                    # spike store back in time order: t = j*CHL1 + c
                    rhs = s5[:, WARM:, b, kt, :].transpose([0, 2, 1])
                    nc.tensor.matmul(
                        v2[:],
                        w2sb[:, kt * H2:(kt + 1) * H2],
                        rhs,
                        start=(kt == 0), stop=(kt == KT2 - 1),
                    )
                p2t = scanp.tile([H2, T], f32, tag="scan", name=f"p2_{b}")
                r2t = scanp.tile([H2, T], f32, tag="scan", name=f"r2_{b}")
                nc.vector.tensor_tensor_scan(
                    p2t[:], dconst[0:H2, :], v2[:], 0.0, op0=MULT, op1=ADD)
                nc.vector.tensor_tensor_scan(
                    r2t[:], dconst[0:H2, :], p2t[:], 0.0, op0=MULT, op1=ADD)
                c0 = b * T
                nc.gpsimd.memset(u2[:, c0:c0 + 1], -THETA)
                nc.scalar.activation(
                    u2[:, c0 + 1:c0 + T], r2t[:, 0:T - 1], COPY,
                    bias=-THETA, scale=CD / CREF)

            # layer-2 spike chain; store keeps cref*s (host divides by cref)
            z2 = l2p.tile([H2, B_PER * NCH2], f32, tag="z2")
            pp2 = l2p.tile([H2, B_PER * NCH2], f32, tag="pp2")
            m2 = l2p.tile([H2, B_PER * NCH2], f32, tag="m2")
            nc.gpsimd.memset(z2[:], 0.0)
            nc.gpsimd.memset(pp2[:], 0.0)
            z2v = z2[:].rearrange("p (b j) -> p b j", b=B_PER)
            pp2v = pp2[:].rearrange("p (b j) -> p b j", b=B_PER)
            m2v = m2[:].rearrange("p (b j) -> p b j", b=B_PER)
            c2v = cref2[:].rearrange("p (b j) -> p b j", b=B_PER)
            u24 = u2[:].rearrange("p (b j c) -> p b j c", b=B_PER, j=NCH2)
            s24 = s2[:].rearrange("p (b j c) -> p b j c", b=B_PER, j=NCH2)
            for i in range(NSTEP2):
                j0, ci = _chunk_slices(i, CHL2)
                zs, ps_ = z2v[:, :, j0:], pp2v[:, :, j0:]
                ms, cs = m2v[:, :, j0:], c2v[:, :, j0:]
                us, ss = u24[:, :, j0:, ci], s24[:, :, j0:, ci]
                nc.vector.scalar_tensor_tensor(zs, zs, D, ps_, op0=MULT, op1=ADD)
                nc.vector.scalar_tensor_tensor(ms, zs, D, us, op0=MULT, op1=ADD)
                nc.vector.scalar_tensor_tensor(ss, ms, 0.0, cs, op0=IS_GE, op1=MULT)
                nc.vector.scalar_tensor_tensor(ps_, ps_, D, ss, op0=MULT, op1=ADD)

            nc.sync.dma_start(y_d[:], s2[:])

    nc.compile()
    return nc


def _get_nc():
    if "nc" not in _CACHE:
        _CACHE["nc"] = _build()
    return _CACHE["nc"]


def _prep_inputs(downsampled, w1, w2):
    x = np.ascontiguousarray(downsampled.reshape(B_TOT, F_IN, T))
    xpad = np.zeros((B_TOT, F_PAD, T), dtype=E4M3)
    xpad[:, :F_IN] = x.astype(E4M3)          # binary spikes: exact in e4m3
    w1t = np.zeros((F_PAD, H1), dtype=E4M3)
    w1t[:F_IN] = np.ascontiguousarray(w1.T).astype(E4M3)
    w2t = np.ascontiguousarray(
        w2.T.reshape(KT2, 128, H2).transpose(1, 0, 2).reshape(128, KT2 * H2)
    ).astype(BF16)
    return [
        {"x": np.ascontiguousarray(xpad[c * B_PER:(c + 1) * B_PER]),
         "w1t": w1t, "w2t": w2t}
        for c in range(N_CORES)
    ]


def kernel(downsampled: np.ndarray, w1: np.ndarray, w2: np.ndarray) -> np.ndarray:
    from concourse.bass_utils import run_bass_kernel_spmd

    nc = _get_nc()
    in_maps = _prep_inputs(downsampled, w1, w2)
    res = run_bass_kernel_spmd(nc, in_maps, core_ids=list(range(N_CORES)))
    out = np.stack([res.results[c]["y"] for c in range(N_CORES)])  # (8, 20, 2T)
    out = out.reshape(N_CORES, H2, B_PER, T).transpose(0, 2, 1, 3)
    out = out.reshape(B_TOT, H2, T) / np.float32(CREF)   # cref*s -> s (exact)
    return np.ascontiguousarray(out.astype(np.float32))


# revision 19
# speedup vs baseline: 1.9796x; 1.2506x over previous
"""Trainium2 Bass kernel for the SLAYER-style 2-layer spiking encoder.

Pipeline per core (2 batches per core, 8 cores, pure data-parallel over batch):
  fc1 (PE, fp8-e4m3 DoubleRow, k-streamed)  ->  alpha-psp scans (DVE
  tensor_tensor_scan)  ->  membrane epilogue (ACT, c-major layout)  ->
  layer-1 spike chain (DVE, 4 ops/step, both batches in one 320-lane chain,
  20 time chunks x 25 steps + 16-step warmup)  ->  fc2 (PE, strided read of
  the step-major spike store)  ->  alpha-psp scans  ->  layer-2 spike chain
  (50 chunks x 10 + 10-step warmup)  ->  DMA out (host divides by the spike
  scale to recover 0/1 spikes).

Key algebraic facts exploited:
  * alpha_psp is linear and commutes with the feature-contracting matmuls:
    matmul the raw binary spikes (exactly representable in fp8/bf16), filter
    the (T,1024) result instead of (T,6300).
  * alpha_psp = two cascaded one-pole recurrences -> two tensor_tensor_scan
    instructions per tile:  p[t] = d*p[t-1] + v[t];  r[t] = d*r[t-1] + p[t];
    membrane drive  u[t] = c*d*r[t-1] - theta.
  * spike_dyn state decays by e^-1 per step, so time chunks processed in
    parallel SIMD lanes from zero state match the sequential result after a
    short warmup (residual 2e-7 .. 4e-3 vs. decision margins; layer-2 margin
    is ~9.0 so even large perturbations cannot flip the output).
  * spike stores hold V*s with V = -20 = bf16(d*cref'): exactly the value the
    refractory state update needs (Q += V*s), exactly representable, and the
    1/V is folded into the next scan scale / host rescale.  This lets the
    threshold op be a two-scalar tensor_scalar (4x DVE mode) and the membrane
    add a pure tensor_tensor (2x DVE mode); scalar_tensor_tensor (used for
    the two state decays) has no fast mode.

Chain-step recurrence in device variables (Zt = d*Z, Q = d*P of the
reference's scaled states):
    Zt = (Zt * d) + Q          # scalar_tensor_tensor
    M  = Zt + U_step           # tensor_tensor      (U is c-major: contiguous)
    S' = (M >= 0) * V          # tensor_scalar      (written to spike store)
    Q  = (Q * d) + S'          # scalar_tensor_tensor
"""

import os
import numpy as np
import ml_dtypes

DEBUG_DUMP = bool(os.environ.get("K_DEBUG"))   # also emit layer-1 spike store

# ---------------------------------------------------------------- constants
B_TOT = 16
B_PER = 2
N_CORES = 8
T = 500
F_IN = 6300
F_PAD = 6400
H1 = 1024
H2 = 20
KP1 = F_PAD // 256    # 25 fp8 DoubleRow k-pair tiles
OT1 = H1 // 128       # 8
KT2 = H1 // 128       # 8

THETA = 10.0
D = float(np.float32(np.exp(-1.0)))
C = float(np.float32(np.e))
CD = C * D
VSP = -20.0           # stored spike value = bf16-exact d*cref (cref'=-54.3662)

WARM1 = 16
NCH1, CHL1 = 20, 25
NSTEP1 = CHL1 + WARM1         # 41
LAN1 = B_PER * OT1 * NCH1     # 320 chain lanes per partition-row
WARM2 = 10
NCH2, CHL2 = 50, 10
NSTEP2 = CHL2 + WARM2         # 20
LAN2 = B_PER * NCH2           # 100

BF16 = ml_dtypes.bfloat16
E4M3 = ml_dtypes.float8_e4m3
_CACHE = {}


def _chunk_slices(i, chl, warm):
    """(first active chunk j0, in-chunk column c) for chain step i."""
    t0 = i - warm
    j0 = 0 if t0 >= 0 else (-t0 + chl - 1) // chl
    return j0, t0 + j0 * chl


def _build():
    import concourse.bass as bass
    import concourse.bacc as bacc
    import concourse.mybir as mybir
    import concourse.tile as tile

    f32 = mybir.dt.float32
    bf16 = mybir.dt.bfloat16
    fp8 = mybir.dt.float8e4
    MULT = mybir.AluOpType.mult
    ADD = mybir.AluOpType.add
    IS_GE = mybir.AluOpType.is_ge
    COPY = mybir.ActivationFunctionType.Copy
    DROW = mybir.MatmulPerfMode.DoubleRow

    nc = bacc.Bacc("TRN2", target_bir_lowering=False, debug=False,
                   num_devices=N_CORES)

    x_d = nc.dram_tensor("x", [B_PER, F_PAD, T], fp8, kind="ExternalInput").ap()
    w1t_d = nc.dram_tensor("w1t", [F_PAD, H1], fp8, kind="ExternalInput").ap()
    w2t_d = nc.dram_tensor("w2t", [128, KT2 * H2], bf16, kind="ExternalInput").ap()
    y_d = nc.dram_tensor("y", [H2, NSTEP2 * LAN2], bf16,
                         kind="ExternalOutput").ap()
    s1_d = (nc.dram_tensor("s1dump", [128, NSTEP1 * LAN1], bf16,
                           kind="ExternalOutput").ap() if DEBUG_DUMP else None)

    with tile.TileContext(nc) as tc:
        with (
            tc.tile_pool(name="xs", bufs=4) as xsp,
            tc.tile_pool(name="w1k", bufs=4) as w1kp,
            tc.tile_pool(name="wee", bufs=1) as wee,
            tc.tile_pool(name="ust", bufs=1) as ustp,
            tc.tile_pool(name="sst", bufs=1) as sstp,
            tc.tile_pool(name="scan", bufs=6) as scanp,
            tc.tile_pool(name="cst", bufs=1) as cstp,
            tc.tile_pool(name="state", bufs=3) as statep,
            tc.tile_pool(name="l2", bufs=1) as l2p,
            tc.tile_pool(name="ps", bufs=8, space="PSUM") as psp,
        ):
            dconst = cstp.tile([128, T], f32, tag="dconst")
            nc.gpsimd.memset(dconst[:], D)
            w2sb = wee.tile([128, KT2 * H2], bf16, tag="w2sb")
            nc.sync.dma_start(w2sb[:], w2t_d[:])

            # c-major membrane store: col = c*LAN1 + b*160 + g*20 + j,
            # holding U[t = j*CHL1 + c] = c*d*r[t-1] - theta  (bf16)
            u_cm = ustp.tile([128, CHL1 * LAN1], bf16, tag="ust")
            u5 = u_cm[:].rearrange("p (c b g j) -> p c b g j",
                                   c=CHL1, b=B_PER, g=OT1)
            nc.gpsimd.memset(u5[:, 0, :, :, 0], -THETA)   # t = 0
            # step-major spike store (contiguous per chain step)
            s_st = sstp.tile([128, NSTEP1 * LAN1], bf16, tag="sst")
            s5 = s_st[:].rearrange("p (i b g j) -> p i b g j",
                                   i=NSTEP1, b=B_PER, g=OT1)

            # ============== per-batch fc1 + scans + membrane epilogue
            for b in range(B_PER):
                xsrc = x_d[b].rearrange("(kp s p) t -> p kp s t", s=2, p=128)
                wsrc = w1t_d.rearrange("(kp s p) o -> p kp s o", s=2, p=128)
                v1 = [psp.tile([128, T], f32, tag="ps", name=f"v1_{b}_{ot}")
                      for ot in range(OT1)]
                for kp in range(KP1):
                    xk = xsp.tile([128, 2 * T], fp8, tag="xs", name=f"x_{b}_{kp}")
                    nc.sync.dma_start(
                        xk[:].rearrange("p (s t) -> p s t", s=2), xsrc[:, kp])
                    w1k = w1kp.tile([128, 2 * H1], fp8, tag="w1k",
                                    name=f"w1_{b}_{kp}")
                    nc.sync.dma_start(
                        w1k[:].rearrange("p (s o) -> p s o", s=2), wsrc[:, kp])
                    w3 = w1k[:].rearrange("p (s o) -> p s o", s=2)
                    x3 = xk[:].rearrange("p (s t) -> p s t", s=2)
                    for ot in range(OT1):
                        nc.tensor.matmul(
                            v1[ot][:],
                            w3[:, :, ot * 128:(ot + 1) * 128],
                            x3,
                            start=(kp == 0), stop=(kp == KP1 - 1),
                            perf_mode=DROW,
                        )

                # alpha-psp scans (DVE) + c-major membrane writes (ACT)
                for ot in range(OT1):
                    p_t = scanp.tile([128, T], f32, tag="scan", name=f"p_{b}_{ot}")
                    r_t = scanp.tile([128, T], f32, tag="scan", name=f"r_{b}_{ot}")
                    nc.vector.tensor_tensor_scan(
                        p_t[:], dconst[:], v1[ot][:], 0.0, op0=MULT, op1=ADD)
                    nc.vector.tensor_tensor_scan(
                        r_t[:], dconst[:], p_t[:], 0.0, op0=MULT, op1=ADD)
                    # chunk j=0, c>=1:  U[t=c] <- cd*r[c-1] - th
                    nc.scalar.activation(
                        u5[:, 1:, b, ot, 0], r_t[:, 0:CHL1 - 1],
                        COPY, bias=-THETA, scale=CD)
                    # chunks j>=1, all c: U[t=j*CHL1+c] <- cd*r[t-1] - th
                    out_ap = u5[:, :, b, ot, 1:].transpose([0, 2, 1])
                    in_ap = (r_t[:, CHL1 - 1:T - 1]
                             .rearrange("p (j c) -> p j c", j=NCH1 - 1))
                    nc.scalar.activation(out_ap, in_ap, COPY,
                                         bias=-THETA, scale=CD)

            # ============== layer-1 spike chain (both batches, 320 lanes)
            zt = statep.tile([128, LAN1], bf16, tag="state", name="z1")
            qt = statep.tile([128, LAN1], bf16, tag="state", name="q1")
            mt = statep.tile([128, LAN1], bf16, tag="state", name="m1")
            nc.gpsimd.memset(zt[:], 0.0)
            nc.gpsimd.memset(qt[:], 0.0)
            z5 = zt[:].rearrange("p (b g j) -> p b g j", b=B_PER, g=OT1)
            q5 = qt[:].rearrange("p (b g j) -> p b g j", b=B_PER, g=OT1)
            m5 = mt[:].rearrange("p (b g j) -> p b g j", b=B_PER, g=OT1)
            for i in range(NSTEP1):
                j0, ci = _chunk_slices(i, CHL1, WARM1)
                if j0 == 0:  # contiguous fast path
                    zs, qs, ms = zt[:], qt[:], mt[:]
                    us = u_cm[:, ci * LAN1:(ci + 1) * LAN1]
                    ss = s_st[:, i * LAN1:(i + 1) * LAN1]
                else:
                    zs, qs = z5[:, :, :, j0:], q5[:, :, :, j0:]
                    ms = m5[:, :, :, j0:]
                    # state slot j warms up on chunk j-j0's history
                    us = u5[:, ci, :, :, 0:NCH1 - j0]
                    ss = s5[:, i, :, :, j0:]
                nc.vector.scalar_tensor_tensor(zs, zs, D, qs, op0=MULT, op1=ADD)
                nc.vector.tensor_tensor(ms, zs, us, op=ADD)
                nc.vector.tensor_scalar(ss, ms, 0.0, VSP, op0=IS_GE, op1=MULT)
                nc.vector.scalar_tensor_tensor(qs, qs, D, ss, op0=MULT, op1=ADD)

            # ============== layer 2
            # c-major membrane/spikes: col = c*LAN2 + b*NCH2 + j,
            # t = j*CHL2 + c
            u2 = l2p.tile([H2, CHL2 * LAN2], bf16, tag="u2")
            s2 = l2p.tile([H2, NSTEP2 * LAN2], bf16, tag="s2")
            u25 = u2[:].rearrange("p (c b j) -> p c b j", c=CHL2, b=B_PER)
            s25 = s2[:].rearrange("p (i b j) -> p i b j", i=NSTEP2, b=B_PER)
            nc.gpsimd.memset(u25[:, 0, :, 0], -THETA)
            for b in range(B_PER):
                v2 = psp.tile([H2, T], f32, tag="ps", name=f"v2_{b}")
                for kt in range(KT2):
                    # spike store back in time order: t = j*CHL1 + c
                    rhs = s5[:, WARM1:, b, kt, :].transpose([0, 2, 1])
                    nc.tensor.matmul(
                        v2[:],
                        w2sb[:, kt * H2:(kt + 1) * H2],
                        rhs,
                        start=(kt == 0), stop=(kt == KT2 - 1),
                    )
                p2t = scanp.tile([H2, T], f32, tag="scan", name=f"p2_{b}")
                r2t = scanp.tile([H2, T], f32, tag="scan", name=f"r2_{b}")
                nc.vector.tensor_tensor_scan(
                    p2t[:], dconst[0:H2, :], v2[:], 0.0, op0=MULT, op1=ADD)
                nc.vector.tensor_tensor_scan(
                    r2t[:], dconst[0:H2, :], p2t[:], 0.0, op0=MULT, op1=ADD)
                nc.scalar.activation(
                    u25[:, 1:, b, 0], r2t[:, 0:CHL2 - 1],
                    COPY, bias=-THETA, scale=CD / VSP)
                out_ap = u25[:, :, b, 1:].transpose([0, 2, 1])
                in_ap = (r2t[:, CHL2 - 1:T - 1]
                         .rearrange("p (j c) -> p j c", j=NCH2 - 1))
                nc.scalar.activation(out_ap, in_ap, COPY,
                                     bias=-THETA, scale=CD / VSP)

            # layer-2 spike chain; store keeps V*s (host divides by V)
            z2 = l2p.tile([H2, LAN2], bf16, tag="z2")
            q2 = l2p.tile([H2, LAN2], bf16, tag="q2")
            m2 = l2p.tile([H2, LAN2], bf16, tag="m2")
            nc.gpsimd.memset(z2[:], 0.0)
            nc.gpsimd.memset(q2[:], 0.0)
            z2v = z2[:].rearrange("p (b j) -> p b j", b=B_PER)
            q2v = q2[:].rearrange("p (b j) -> p b j", b=B_PER)
            m2v = m2[:].rearrange("p (b j) -> p b j", b=B_PER)
            for i in range(NSTEP2):
                j0, ci = _chunk_slices(i, CHL2, WARM2)
                if j0 == 0:
                    zs, qs, ms = z2[:], q2[:], m2[:]
                    us = u2[:, ci * LAN2:(ci + 1) * LAN2]
                    ss = s2[:, i * LAN2:(i + 1) * LAN2]
                else:
                    zs, qs = z2v[:, :, j0:], q2v[:, :, j0:]
                    ms = m2v[:, :, j0:]
                    # state slot j warms up on chunk j-j0's history
                    us = u25[:, ci, :, 0:NCH2 - j0]
                    ss = s25[:, i, :, j0:]
                nc.vector.scalar_tensor_tensor(zs, zs, D, qs, op0=MULT, op1=ADD)
                nc.vector.tensor_tensor(ms, zs, us, op=ADD)
                nc.vector.tensor_scalar(ss, ms, 0.0, VSP, op0=IS_GE, op1=MULT)
                nc.vector.scalar_tensor_tensor(qs, qs, D, ss, op0=MULT, op1=ADD)

            # ship the whole step-major spike store; host extracts output phase
            nc.sync.dma_start(y_d[:], s2[:])
            if DEBUG_DUMP:
                nc.sync.dma_start(s1_d[:], s_st[:])

    nc.compile()
    return nc


def _get_nc():
    if "nc" not in _CACHE:
        _CACHE["nc"] = _build()
    return _CACHE["nc"]


def _prep_inputs(downsampled, w1, w2):
    x = np.ascontiguousarray(downsampled.reshape(B_TOT, F_IN, T))
    xpad = np.zeros((B_TOT, F_PAD, T), dtype=E4M3)
    xpad[:, :F_IN] = x.astype(E4M3)          # binary spikes: exact in e4m3
    w1t = np.zeros((F_PAD, H1), dtype=E4M3)
    w1t[:F_IN] = np.ascontiguousarray(w1.T).astype(E4M3)
    w2t = np.ascontiguousarray(
        w2.T.reshape(KT2, 128, H2).transpose(1, 0, 2).reshape(128, KT2 * H2)
    ).astype(BF16)
    return [
        {"x": np.ascontiguousarray(xpad[c * B_PER:(c + 1) * B_PER]),
         "w1t": w1t, "w2t": w2t}
        for c in range(N_CORES)
    ]


def kernel(downsampled: np.ndarray, w1: np.ndarray, w2: np.ndarray) -> np.ndarray:
    from concourse.bass_utils import run_bass_kernel_spmd

    nc = _get_nc()
    in_maps = _prep_inputs(downsampled, w1, w2)
    res = run_bass_kernel_spmd(nc, in_maps, core_ids=list(range(N_CORES)))
    out = np.stack([res.results[c]["y"] for c in range(N_CORES)])
    # y is the step-major spike store: [o2, (i b j)]; output phase i>=WARM2
    # holds spikes for t = j*CHL2 + (i - WARM2), scaled by V.
    out = out.reshape(N_CORES, H2, NSTEP2, B_PER, NCH2).astype(np.float32)
    out = out[:, :, WARM2:]                      # (core, o2, c, b, j)
    out = out.transpose(0, 3, 1, 4, 2)           # core, b, o2, j, c
    out = out.reshape(B_TOT, H2, T) / np.float32(VSP)   # V*s -> s (exact)
    return np.ascontiguousarray(out.astype(np.float32))


# revision 24
# speedup vs baseline: 2.1441x; 1.0831x over previous
"""Trainium2 Bass kernel for the SLAYER-style 2-layer spiking encoder.

Pipeline per core (2 batches per core, 8 cores, pure data-parallel over batch):
  fc1 (PE, fp8-e4m3 DoubleRow, k-streamed)  ->  alpha-psp scans (DVE
  tensor_tensor_scan)  ->  membrane epilogue (ACT, c-major layout)  ->
  layer-1 spike chain (DVE, 4 ops/step, both batches in one 320-lane chain,
  20 time chunks x 25 steps + 16-step warmup)  ->  fc2 (PE, strided read of
  the step-major spike store)  ->  alpha-psp scans  ->  layer-2 spike chain
  (50 chunks x 10 + 10-step warmup)  ->  DMA out (host divides by the spike
  scale to recover 0/1 spikes).

Key algebraic facts exploited:
  * alpha_psp is linear and commutes with the feature-contracting matmuls:
    matmul the raw binary spikes (exactly representable in fp8/bf16), filter
    the (T,1024) result instead of (T,6300).
  * alpha_psp = two cascaded one-pole recurrences -> two tensor_tensor_scan
    instructions per tile:  p[t] = d*p[t-1] + v[t];  r[t] = d*r[t-1] + p[t];
    membrane drive  u[t] = c*d*r[t-1] - theta.
  * spike_dyn state decays by e^-1 per step, so time chunks processed in
    parallel SIMD lanes from zero state match the sequential result after a
    short warmup (residual 2e-7 .. 4e-3 vs. decision margins; layer-2 margin
    is ~9.0 so even large perturbations cannot flip the output).
  * spike stores hold V*s with V = -20 = bf16(d*cref'): exactly the value the
    refractory state update needs (Q += V*s), exactly representable, and the
    1/V is folded into the next scan scale / host rescale.  This lets the
    threshold op be a two-scalar tensor_scalar (4x DVE mode) and the membrane
    add a pure tensor_tensor (2x DVE mode); scalar_tensor_tensor (used for
    the two state decays) has no fast mode.

Chain-step recurrence in device variables (Zt = d*Z, Q = d*P of the
reference's scaled states):
    Zt = (Zt * d) + Q          # scalar_tensor_tensor
    M  = Zt + U_step           # tensor_tensor      (U is c-major: contiguous)
    S' = (M >= 0) * V          # tensor_scalar      (written to spike store)
    Q  = (Q * d) + S'          # scalar_tensor_tensor
"""

import os
import numpy as np
import ml_dtypes

DEBUG_DUMP = bool(os.environ.get("K_DEBUG"))   # also emit layer-1 spike store

# ---------------------------------------------------------------- constants
B_TOT = 16
B_PER = 2
N_CORES = 8
T = 500
F_IN = 6300
F_PAD = 6400
H1 = 1024
H2 = 20
KP1 = F_PAD // 256    # 25 fp8 DoubleRow k-pair tiles
OT1 = H1 // 128       # 8
KT2 = H1 // 128       # 8

THETA = 10.0
D = float(np.float32(np.exp(-1.0)))
C = float(np.float32(np.e))
CD = C * D
VSP = -20.0           # stored spike value = bf16-exact d*cref (cref'=-54.3662)

WARM1 = 16
NCH1, CHL1 = 20, 25
NSTEP1 = CHL1 + WARM1         # 41
LAN1 = B_PER * OT1 * NCH1     # 320 chain lanes per partition-row
WARM2 = 10
NCH2, CHL2 = 50, 10
NSTEP2 = CHL2 + WARM2         # 20
LAN2 = B_PER * NCH2           # 100

BF16 = ml_dtypes.bfloat16
E4M3 = ml_dtypes.float8_e4m3
_CACHE = {}


def _chunk_slices(i, chl, warm):
    """(first active chunk j0, in-chunk column c) for chain step i."""
    t0 = i - warm
    j0 = 0 if t0 >= 0 else (-t0 + chl - 1) // chl
    return j0, t0 + j0 * chl


def _build():
    import concourse.bass as bass
    import concourse.bacc as bacc
    import concourse.mybir as mybir
    import concourse.tile as tile

    f32 = mybir.dt.float32
    bf16 = mybir.dt.bfloat16
    fp8 = mybir.dt.float8e4
    MULT = mybir.AluOpType.mult
    ADD = mybir.AluOpType.add
    IS_GE = mybir.AluOpType.is_ge
    COPY = mybir.ActivationFunctionType.Copy
    DROW = mybir.MatmulPerfMode.DoubleRow

    nc = bacc.Bacc("TRN2", target_bir_lowering=False, debug=False,
                   num_devices=N_CORES)

    x_d = nc.dram_tensor("x", [B_PER, F_PAD, T], fp8, kind="ExternalInput").ap()
    # host-permuted weights: [ot][kp][s][p][o] so one o-tile = one linear DMA
    w1t_d = nc.dram_tensor("w1t", [OT1, KP1 * 2 * 128 * 128], fp8,
                           kind="ExternalInput").ap()
    w2t_d = nc.dram_tensor("w2t", [128, KT2 * H2], bf16, kind="ExternalInput").ap()
    y_d = nc.dram_tensor("y", [H2, NSTEP2 * LAN2], bf16,
                         kind="ExternalOutput").ap()
    s1_d = (nc.dram_tensor("s1dump", [128, NSTEP1 * LAN1], bf16,
                           kind="ExternalOutput").ap() if DEBUG_DUMP else None)

    with tile.TileContext(nc) as tc:
        with (
            tc.tile_pool(name="xs", bufs=4) as xsp,
            tc.tile_pool(name="w1k", bufs=4) as w1kp,
            tc.tile_pool(name="wee", bufs=1) as wee,
            tc.tile_pool(name="ust", bufs=1) as ustp,
            tc.tile_pool(name="sst", bufs=1) as sstp,
            tc.tile_pool(name="scan", bufs=6) as scanp,
            tc.tile_pool(name="cst", bufs=1) as cstp,
            tc.tile_pool(name="state", bufs=3) as statep,
            tc.tile_pool(name="l2", bufs=1) as l2p,
            tc.tile_pool(name="ps", bufs=8, space="PSUM") as psp,
        ):
            dconst = cstp.tile([128, T], f32, tag="dconst")
            nc.gpsimd.memset(dconst[:], D)
            w2sb = wee.tile([128, KT2 * H2], bf16, tag="w2sb")
            nc.sync.dma_start(w2sb[:], w2t_d[:])

            # c-major membrane store: col = c*LAN1 + b*160 + g*20 + j,
            # holding U[t = j*CHL1 + c] = c*d*r[t-1] - theta  (bf16)
            u_cm = ustp.tile([128, CHL1 * LAN1], bf16, tag="ust")
            u5 = u_cm[:].rearrange("p (c b g j) -> p c b g j",
                                   c=CHL1, b=B_PER, g=OT1)
            nc.gpsimd.memset(u5[:, 0, :, :, 0], -THETA)   # t = 0
            # step-major spike store (contiguous per chain step)
            s_st = sstp.tile([128, NSTEP1 * LAN1], bf16, tag="sst")
            s5 = s_st[:].rearrange("p (i b g j) -> p i b g j",
                                   i=NSTEP1, b=B_PER, g=OT1)
            # layer-1 chain state (warmup phases run per batch, interleaved
            # with the other batch's fc1; main phase runs all 320 lanes)
            zt = statep.tile([128, LAN1], bf16, tag="state", name="z1")
            qt = statep.tile([128, LAN1], bf16, tag="state", name="q1")
            mt = statep.tile([128, LAN1], bf16, tag="state", name="m1")
            nc.gpsimd.memset(zt[:], 0.0)
            nc.gpsimd.memset(qt[:], 0.0)
            z5 = zt[:].rearrange("p (b g j) -> p b g j", b=B_PER, g=OT1)
            q5 = qt[:].rearrange("p (b g j) -> p b g j", b=B_PER, g=OT1)
            m5 = mt[:].rearrange("p (b g j) -> p b g j", b=B_PER, g=OT1)

            # ============== per-batch fc1 + scans + membrane epilogue
            # o-major: x resident per batch, one PSUM bank at a time, so each
            # o-tile's scans/epilogue pipeline inside fc1.
            for b in range(B_PER):
                xr = xsp.tile([128, KP1 * 2 * T], fp8, tag="xs", name=f"x_{b}")
                nc.sync.dma_start(
                    xr[:].rearrange("p (kp s t) -> p kp s t", kp=KP1, s=2),
                    x_d[b].rearrange("(kp s p) t -> p kp s t", s=2, p=128))
                x4 = xr[:].rearrange("p (kp s t) -> p kp s t", kp=KP1, s=2)
                for ot in range(OT1):
                    w1o = w1kp.tile([128, KP1 * 2 * 128], fp8, tag="w1k",
                                    name=f"w1_{b}_{ot}")
                    nc.sync.dma_start(
                        w1o[:].rearrange("p (kp s o) -> p kp s o", kp=KP1, s=2),
                        w1t_d[ot].rearrange("(kp s p o) -> p kp s o",
                                            kp=KP1, s=2, p=128))
                    w4 = w1o[:].rearrange("p (kp s o) -> p kp s o", kp=KP1, s=2)
                    v1 = psp.tile([128, T], f32, tag="ps", name=f"v1_{b}_{ot}")
                    for kp in range(KP1):
                        nc.tensor.matmul(
                            v1[:], w4[:, kp], x4[:, kp],
                            start=(kp == 0), stop=(kp == KP1 - 1),
                            perf_mode=DROW,
                        )
                    # alpha-psp scans (DVE) + c-major membrane writes (ACT)
                    p_t = scanp.tile([128, T], f32, tag="scan", name=f"p_{b}_{ot}")
                    r_t = scanp.tile([128, T], f32, tag="scan", name=f"r_{b}_{ot}")
                    nc.vector.tensor_tensor_scan(
                        p_t[:], dconst[:], v1[:], 0.0, op0=MULT, op1=ADD)
                    nc.vector.tensor_tensor_scan(
                        r_t[:], dconst[:], p_t[:], 0.0, op0=MULT, op1=ADD)
                    # chunk j=0, c>=1:  U[t=c] <- cd*r[c-1] - th
                    nc.scalar.activation(
                        u5[:, 1:, b, ot, 0], r_t[:, 0:CHL1 - 1],
                        COPY, bias=-THETA, scale=CD)
                    # chunks j>=1, all c: U[t=j*CHL1+c] <- cd*r[t-1] - th
                    out_ap = u5[:, :, b, ot, 1:].transpose([0, 2, 1])
                    in_ap = (r_t[:, CHL1 - 1:T - 1]
                             .rearrange("p (j c) -> p j c", j=NCH1 - 1))
                    nc.scalar.activation(out_ap, in_ap, COPY,
                                         bias=-THETA, scale=CD)

                # batch-b chain warmup: b=0's fills the DVE idle window while
                # the PE runs batch 1's fc1 (touches only this batch's lanes)
                bs = slice(b, b + 1)
                for i in range(WARM1):
                    _, ci = _chunk_slices(i, CHL1, WARM1)
                    zs, qs = z5[:, bs, :, 1:], q5[:, bs, :, 1:]
                    ms = m5[:, bs, :, 1:]
                    us = u5[:, ci, bs, :, 0:NCH1 - 1]
                    ss = s5[:, i, bs, :, 1:]
                    nc.vector.scalar_tensor_tensor(zs, zs, D, qs, op0=MULT, op1=ADD)
                    nc.vector.tensor_tensor(ms, zs, us, op=ADD)
                    nc.vector.tensor_scalar(ss, ms, 0.0, VSP, op0=IS_GE, op1=MULT)
                    nc.vector.scalar_tensor_tensor(qs, qs, D, ss, op0=MULT, op1=ADD)

            # ============== layer-1 spike chain main phase (320 lanes)
            for i in range(WARM1, NSTEP1):
                ci = i - WARM1
                zs, qs, ms = zt[:], qt[:], mt[:]
                us = u_cm[:, ci * LAN1:(ci + 1) * LAN1]
                ss = s_st[:, i * LAN1:(i + 1) * LAN1]
                nc.vector.scalar_tensor_tensor(zs, zs, D, qs, op0=MULT, op1=ADD)
                nc.vector.tensor_tensor(ms, zs, us, op=ADD)
                nc.vector.tensor_scalar(ss, ms, 0.0, VSP, op0=IS_GE, op1=MULT)
                nc.vector.scalar_tensor_tensor(qs, qs, D, ss, op0=MULT, op1=ADD)

            # ============== layer 2
            # c-major membrane/spikes: col = c*LAN2 + b*NCH2 + j,
            # t = j*CHL2 + c
            u2 = l2p.tile([H2, CHL2 * LAN2], bf16, tag="u2")
            s2 = l2p.tile([H2, NSTEP2 * LAN2], bf16, tag="s2")
            u25 = u2[:].rearrange("p (c b j) -> p c b j", c=CHL2, b=B_PER)
            s25 = s2[:].rearrange("p (i b j) -> p i b j", i=NSTEP2, b=B_PER)
            nc.gpsimd.memset(u25[:, 0, :, 0], -THETA)
            for b in range(B_PER):
                v2 = psp.tile([H2, T], f32, tag="ps", name=f"v2_{b}")
                for kt in range(KT2):
                    # spike store back in time order: t = j*CHL1 + c
                    rhs = s5[:, WARM1:, b, kt, :].transpose([0, 2, 1])
                    nc.tensor.matmul(
                        v2[:],
                        w2sb[:, kt * H2:(kt + 1) * H2],
                        rhs,
                        start=(kt == 0), stop=(kt == KT2 - 1),
                    )
                p2t = scanp.tile([H2, T], f32, tag="scan", name=f"p2_{b}")
                r2t = scanp.tile([H2, T], f32, tag="scan", name=f"r2_{b}")
                nc.vector.tensor_tensor_scan(
                    p2t[:], dconst[0:H2, :], v2[:], 0.0, op0=MULT, op1=ADD)
                nc.vector.tensor_tensor_scan(
                    r2t[:], dconst[0:H2, :], p2t[:], 0.0, op0=MULT, op1=ADD)
                nc.scalar.activation(
                    u25[:, 1:, b, 0], r2t[:, 0:CHL2 - 1],
                    COPY, bias=-THETA, scale=CD / VSP)
                out_ap = u25[:, :, b, 1:].transpose([0, 2, 1])
                in_ap = (r2t[:, CHL2 - 1:T - 1]
                         .rearrange("p (j c) -> p j c", j=NCH2 - 1))
                nc.scalar.activation(out_ap, in_ap, COPY,
                                     bias=-THETA, scale=CD / VSP)

            # layer-2 spike chain; store keeps V*s (host divides by V)
            z2 = l2p.tile([H2, LAN2], bf16, tag="z2")
            q2 = l2p.tile([H2, LAN2], bf16, tag="q2")
            m2 = l2p.tile([H2, LAN2], bf16, tag="m2")
            nc.gpsimd.memset(z2[:], 0.0)
            nc.gpsimd.memset(q2[:], 0.0)
            z2v = z2[:].rearrange("p (b j) -> p b j", b=B_PER)
            q2v = q2[:].rearrange("p (b j) -> p b j", b=B_PER)
            m2v = m2[:].rearrange("p (b j) -> p b j", b=B_PER)
            for i in range(NSTEP2):
                j0, ci = _chunk_slices(i, CHL2, WARM2)
                if j0 == 0:
                    zs, qs, ms = z2[:], q2[:], m2[:]
                    us = u2[:, ci * LAN2:(ci + 1) * LAN2]
                    ss = s2[:, i * LAN2:(i + 1) * LAN2]
                else:
                    zs, qs = z2v[:, :, j0:], q2v[:, :, j0:]
                    ms = m2v[:, :, j0:]
                    # state slot j warms up on chunk j-j0's history
                    us = u25[:, ci, :, 0:NCH2 - j0]
                    ss = s25[:, i, :, j0:]
                nc.vector.scalar_tensor_tensor(zs, zs, D, qs, op0=MULT, op1=ADD)
                nc.vector.tensor_tensor(ms, zs, us, op=ADD)
                nc.vector.tensor_scalar(ss, ms, 0.0, VSP, op0=IS_GE, op1=MULT)
                nc.vector.scalar_tensor_tensor(qs, qs, D, ss, op0=MULT, op1=ADD)

            # ship the whole step-major spike store; host extracts output phase
            nc.sync.dma_start(y_d[:], s2[:])
            if DEBUG_DUMP:
                nc.sync.dma_start(s1_d[:], s_st[:])

    nc.compile()
    return nc


def _get_nc():
    if "nc" not in _CACHE:
        _CACHE["nc"] = _build()
    return _CACHE["nc"]


def _prep_inputs(downsampled, w1, w2):
    x = np.ascontiguousarray(downsampled.reshape(B_TOT, F_IN, T))
    xpad = np.zeros((B_TOT, F_PAD, T), dtype=E4M3)
    xpad[:, :F_IN] = x.astype(E4M3)          # binary spikes: exact in e4m3
    w1t = np.zeros((F_PAD, H1), dtype=E4M3)
    w1t[:F_IN] = np.ascontiguousarray(w1.T).astype(E4M3)
    # [f, o] -> [ot][kp][s][p][o_local] so one o-tile is a linear DMA
    w1t = np.ascontiguousarray(
        w1t.reshape(KP1, 2, 128, OT1, 128).transpose(3, 0, 1, 2, 4)
        .reshape(OT1, KP1 * 2 * 128 * 128))
    w2t = np.ascontiguousarray(
        w2.T.reshape(KT2, 128, H2).transpose(1, 0, 2).reshape(128, KT2 * H2)
    ).astype(BF16)
    return [
        {"x": np.ascontiguousarray(xpad[c * B_PER:(c + 1) * B_PER]),
         "w1t": w1t, "w2t": w2t}
        for c in range(N_CORES)
    ]


def kernel(downsampled: np.ndarray, w1: np.ndarray, w2: np.ndarray) -> np.ndarray:
    from concourse.bass_utils import run_bass_kernel_spmd

    nc = _get_nc()
    in_maps = _prep_inputs(downsampled, w1, w2)
    res = run_bass_kernel_spmd(nc, in_maps, core_ids=list(range(N_CORES)))
    out = np.stack([res.results[c]["y"] for c in range(N_CORES)])
    # y is the step-major spike store: [o2, (i b j)]; output phase i>=WARM2
    # holds spikes for t = j*CHL2 + (i - WARM2), scaled by V.
    out = out.reshape(N_CORES, H2, NSTEP2, B_PER, NCH2).astype(np.float32)
    out = out[:, :, WARM2:]                      # (core, o2, c, b, j)
    out = out.transpose(0, 3, 1, 4, 2)           # core, b, o2, j, c
    out = out.reshape(B_TOT, H2, T) / np.float32(VSP)   # V*s -> s (exact)
    return np.ascontiguousarray(out.astype(np.float32))


# revision 27
# speedup vs baseline: 2.1993x; 1.0258x over previous
"""Trainium2 Bass kernel for the SLAYER-style 2-layer spiking encoder.

Pipeline per core (2 batches per core, 8 cores, pure data-parallel over batch):
  fc1 (PE, fp8-e4m3 DoubleRow, k-streamed)  ->  alpha-psp scans (DVE
  tensor_tensor_scan)  ->  membrane epilogue (ACT, c-major layout)  ->
  layer-1 spike chain (DVE, 4 ops/step, both batches in one 320-lane chain,
  20 time chunks x 25 steps + 16-step warmup)  ->  fc2 (PE, strided read of
  the step-major spike store)  ->  alpha-psp scans  ->  layer-2 spike chain
  (50 chunks x 10 + 10-step warmup)  ->  DMA out (host divides by the spike
  scale to recover 0/1 spikes).

Key algebraic facts exploited:
  * alpha_psp is linear and commutes with the feature-contracting matmuls:
    matmul the raw binary spikes (exactly representable in fp8/bf16), filter
    the (T,1024) result instead of (T,6300).
  * alpha_psp = two cascaded one-pole recurrences -> two tensor_tensor_scan
    instructions per tile:  p[t] = d*p[t-1] + v[t];  r[t] = d*r[t-1] + p[t];
    membrane drive  u[t] = c*d*r[t-1] - theta.
  * spike_dyn state decays by e^-1 per step, so time chunks processed in
    parallel SIMD lanes from zero state match the sequential result after a
    short warmup (residual 2e-7 .. 4e-3 vs. decision margins; layer-2 margin
    is ~9.0 so even large perturbations cannot flip the output).
  * spike stores hold V*s with V = -20 = bf16(d*cref'): exactly the value the
    refractory state update needs (Q += V*s), exactly representable, and the
    1/V is folded into the next scan scale / host rescale.  This lets the
    threshold op be a two-scalar tensor_scalar (4x DVE mode) and the membrane
    add a pure tensor_tensor (2x DVE mode); scalar_tensor_tensor (used for
    the two state decays) has no fast mode.

Chain-step recurrence in device variables (Zt = d*Z, Q = d*P of the
reference's scaled states):
    Zt = (Zt * d) + Q          # scalar_tensor_tensor
    M  = Zt + U_step           # tensor_tensor      (U is c-major: contiguous)
    S' = (M >= 0) * V          # tensor_scalar      (written to spike store)
    Q  = (Q * d) + S'          # scalar_tensor_tensor
"""

import os
import numpy as np
import ml_dtypes

DEBUG_DUMP = bool(os.environ.get("K_DEBUG"))   # also emit layer-1 spike store

# ---------------------------------------------------------------- constants
B_TOT = 16
B_PER = 2
N_CORES = 8
T = 500
F_IN = 6300
F_PAD = 6400
H1 = 1024
H2 = 20
KP1 = F_PAD // 256    # 25 fp8 DoubleRow k-pair tiles
OT1 = H1 // 128       # 8
KT2 = H1 // 128       # 8

THETA = 10.0
D = float(np.float32(np.exp(-1.0)))
C = float(np.float32(np.e))
CD = C * D
VSP = -20.0           # stored spike value = bf16-exact d*cref (cref'=-54.3662)

WARM1 = 12
NCH1, CHL1 = 25, 20
NSTEP1 = CHL1 + WARM1         # 32
LAN1 = B_PER * OT1 * NCH1     # 400 chain lanes per partition-row
WARM2 = 10
NCH2, CHL2 = 50, 10
NSTEP2 = CHL2 + WARM2         # 20
LAN2 = B_PER * NCH2           # 100

BF16 = ml_dtypes.bfloat16
E4M3 = ml_dtypes.float8_e4m3
_CACHE = {}


def _chunk_slices(i, chl, warm):
    """(first active chunk j0, in-chunk column c) for chain step i."""
    t0 = i - warm
    j0 = 0 if t0 >= 0 else (-t0 + chl - 1) // chl
    return j0, t0 + j0 * chl


def _build():
    import concourse.bass as bass
    import concourse.bacc as bacc
    import concourse.mybir as mybir
    import concourse.tile as tile

    f32 = mybir.dt.float32
    bf16 = mybir.dt.bfloat16
    fp8 = mybir.dt.float8e4
    MULT = mybir.AluOpType.mult
    ADD = mybir.AluOpType.add
    IS_GE = mybir.AluOpType.is_ge
    COPY = mybir.ActivationFunctionType.Copy
    DROW = mybir.MatmulPerfMode.DoubleRow

    nc = bacc.Bacc("TRN2", target_bir_lowering=False, debug=False,
                   num_devices=N_CORES)

    x_d = nc.dram_tensor("x", [B_PER, F_PAD, T], fp8, kind="ExternalInput").ap()
    # host-permuted weights: [ot][kp][s][p][o] so one o-tile = one linear DMA
    w1t_d = nc.dram_tensor("w1t", [OT1, KP1 * 2 * 128 * 128], fp8,
                           kind="ExternalInput").ap()
    w2t_d = nc.dram_tensor("w2t", [128, KT2 * H2], bf16, kind="ExternalInput").ap()
    y_d = nc.dram_tensor("y", [H2, NSTEP2 * LAN2], bf16,
                         kind="ExternalOutput").ap()
    s1_d = (nc.dram_tensor("s1dump", [128, NSTEP1 * LAN1], bf16,
                           kind="ExternalOutput").ap() if DEBUG_DUMP else None)

    with tile.TileContext(nc) as tc:
        with (
            tc.tile_pool(name="xs", bufs=2) as xsp,
            tc.tile_pool(name="w1k", bufs=3) as w1kp,
            tc.tile_pool(name="wee", bufs=1) as wee,
            tc.tile_pool(name="ust", bufs=1) as ustp,
            tc.tile_pool(name="sst", bufs=1) as sstp,
            tc.tile_pool(name="scan", bufs=6) as scanp,
            tc.tile_pool(name="cst", bufs=1) as cstp,
            tc.tile_pool(name="state", bufs=3) as statep,
            tc.tile_pool(name="l2", bufs=1) as l2p,
            tc.tile_pool(name="ps", bufs=8, space="PSUM") as psp,
        ):
            dconst = cstp.tile([128, T], f32, tag="dconst")
            nc.gpsimd.memset(dconst[:], D)
            w2sb = wee.tile([128, KT2 * H2], bf16, tag="w2sb")
            nc.sync.dma_start(w2sb[:], w2t_d[:])

            # c-major membrane store: col = c*LAN1 + b*160 + g*20 + j,
            # holding U[t = j*CHL1 + c] = c*d*r[t-1] - theta  (bf16)
            u_cm = ustp.tile([128, CHL1 * LAN1], bf16, tag="ust")
            u5 = u_cm[:].rearrange("p (c b g j) -> p c b g j",
                                   c=CHL1, b=B_PER, g=OT1)
            nc.gpsimd.memset(u5[:, 0, :, :, 0], -THETA)   # t = 0
            # step-major spike store (contiguous per chain step)
            s_st = sstp.tile([128, NSTEP1 * LAN1], bf16, tag="sst")
            s5 = s_st[:].rearrange("p (i b g j) -> p i b g j",
                                   i=NSTEP1, b=B_PER, g=OT1)
            # layer-1 chain state (warmup phases run per batch, interleaved
            # with the other batch's fc1; main phase runs all 320 lanes)
            zt = statep.tile([128, LAN1], bf16, tag="state", name="z1")
            qt = statep.tile([128, LAN1], bf16, tag="state", name="q1")
            mt = statep.tile([128, LAN1], bf16, tag="state", name="m1")
            nc.gpsimd.memset(zt[:], 0.0)
            nc.gpsimd.memset(qt[:], 0.0)
            z5 = zt[:].rearrange("p (b g j) -> p b g j", b=B_PER, g=OT1)
            q5 = qt[:].rearrange("p (b g j) -> p b g j", b=B_PER, g=OT1)
            m5 = mt[:].rearrange("p (b g j) -> p b g j", b=B_PER, g=OT1)

            # ============== per-batch fc1 + scans + membrane epilogue
            # o-major: x resident per batch, one PSUM bank at a time, so each
            # o-tile's scans/epilogue pipeline inside fc1.
            xtiles = []
            for b in range(B_PER):
                xr = xsp.tile([128, KP1 * 2 * T], fp8, tag="xs", name=f"x_{b}")
                xv = xr[:].rearrange("p (kp s t) -> p kp s t", kp=KP1, s=2)
                src = x_d[b].rearrange("(kp s p) t -> p kp s t", s=2, p=128)
                for j in range(0, KP1, 5):
                    nc.sync.dma_start(xv[:, j:j + 5], src[:, j:j + 5])
                xtiles.append(xv)
            for b in range(B_PER):
                x4 = xtiles[b]
                for ot in range(OT1):
                    w1o = w1kp.tile([128, KP1 * 2 * 128], fp8, tag="w1k",
                                    name=f"w1_{b}_{ot}")
                    nc.sync.dma_start(
                        w1o[:].rearrange("p (kp s o) -> p kp s o", kp=KP1, s=2),
                        w1t_d[ot].rearrange("(kp s p o) -> p kp s o",
                                            kp=KP1, s=2, p=128))
                    w4 = w1o[:].rearrange("p (kp s o) -> p kp s o", kp=KP1, s=2)
                    v1 = psp.tile([128, T], f32, tag="ps", name=f"v1_{b}_{ot}")
                    for kp in range(KP1):
                        nc.tensor.matmul(
                            v1[:], w4[:, kp], x4[:, kp],
                            start=(kp == 0), stop=(kp == KP1 - 1),
                            perf_mode=DROW,
                        )
                    # alpha-psp scans (DVE) + c-major membrane writes (ACT)
                    p_t = scanp.tile([128, T], f32, tag="scan", name=f"p_{b}_{ot}")
                    r_t = scanp.tile([128, T], f32, tag="scan", name=f"r_{b}_{ot}")
                    nc.vector.tensor_tensor_scan(
                        p_t[:], dconst[:], v1[:], 0.0, op0=MULT, op1=ADD)
                    nc.vector.tensor_tensor_scan(
                        r_t[:], dconst[:], p_t[:], 0.0, op0=MULT, op1=ADD)
                    # chunk j=0, c>=1:  U[t=c] <- cd*r[c-1] - th
                    nc.scalar.activation(
                        u5[:, 1:, b, ot, 0], r_t[:, 0:CHL1 - 1],
                        COPY, bias=-THETA, scale=CD)
                    # chunks j>=1, all c: U[t=j*CHL1+c] <- cd*r[t-1] - th
                    out_ap = u5[:, :, b, ot, 1:].transpose([0, 2, 1])
                    in_ap = (r_t[:, CHL1 - 1:T - 1]
                             .rearrange("p (j c) -> p j c", j=NCH1 - 1))
                    nc.scalar.activation(out_ap, in_ap, COPY,
                                         bias=-THETA, scale=CD)

                # batch-b chain warmup: b=0's fills the DVE idle window while
                # the PE runs batch 1's fc1 (touches only this batch's lanes)
                bs = slice(b, b + 1)
                for i in range(WARM1):
                    _, ci = _chunk_slices(i, CHL1, WARM1)
                    zs, qs = z5[:, bs, :, 1:], q5[:, bs, :, 1:]
                    ms = m5[:, bs, :, 1:]
                    us = u5[:, ci, bs, :, 0:NCH1 - 1]
                    ss = s5[:, i, bs, :, 1:]
                    nc.vector.scalar_tensor_tensor(zs, zs, D, qs, op0=MULT, op1=ADD)
                    nc.vector.tensor_tensor(ms, zs, us, op=ADD)
                    nc.vector.tensor_scalar(ss, ms, 0.0, VSP, op0=IS_GE, op1=MULT)
                    nc.vector.scalar_tensor_tensor(qs, qs, D, ss, op0=MULT, op1=ADD)

            # ============== layer-1 spike chain main phase (320 lanes)
            for i in range(WARM1, NSTEP1):
                ci = i - WARM1
                zs, qs, ms = zt[:], qt[:], mt[:]
                us = u_cm[:, ci * LAN1:(ci + 1) * LAN1]
                ss = s_st[:, i * LAN1:(i + 1) * LAN1]
                nc.vector.scalar_tensor_tensor(zs, zs, D, qs, op0=MULT, op1=ADD)
                nc.vector.tensor_tensor(ms, zs, us, op=ADD)
                nc.vector.tensor_scalar(ss, ms, 0.0, VSP, op0=IS_GE, op1=MULT)
                nc.vector.scalar_tensor_tensor(qs, qs, D, ss, op0=MULT, op1=ADD)

            # ============== layer 2
            # c-major membrane/spikes: col = c*LAN2 + b*NCH2 + j,
            # t = j*CHL2 + c
            u2 = l2p.tile([H2, CHL2 * LAN2], bf16, tag="u2")
            s2 = l2p.tile([H2, NSTEP2 * LAN2], bf16, tag="s2")
            u25 = u2[:].rearrange("p (c b j) -> p c b j", c=CHL2, b=B_PER)
            s25 = s2[:].rearrange("p (i b j) -> p i b j", i=NSTEP2, b=B_PER)
            nc.gpsimd.memset(u25[:, 0, :, 0], -THETA)
            for b in range(B_PER):
                v2 = psp.tile([H2, T], f32, tag="ps", name=f"v2_{b}")
                for kt in range(KT2):
                    # spike store back in time order: t = j*CHL1 + c
                    rhs = s5[:, WARM1:, b, kt, :].transpose([0, 2, 1])
                    nc.tensor.matmul(
                        v2[:],
                        w2sb[:, kt * H2:(kt + 1) * H2],
                        rhs,
                        start=(kt == 0), stop=(kt == KT2 - 1),
                    )
                p2t = scanp.tile([H2, T], f32, tag="scan", name=f"p2_{b}")
                r2t = scanp.tile([H2, T], f32, tag="scan", name=f"r2_{b}")
                nc.vector.tensor_tensor_scan(
                    p2t[:], dconst[0:H2, :], v2[:], 0.0, op0=MULT, op1=ADD)
                nc.vector.tensor_tensor_scan(
                    r2t[:], dconst[0:H2, :], p2t[:], 0.0, op0=MULT, op1=ADD)
                nc.scalar.activation(
                    u25[:, 1:, b, 0], r2t[:, 0:CHL2 - 1],
                    COPY, bias=-THETA, scale=CD / VSP)
                out_ap = u25[:, :, b, 1:].transpose([0, 2, 1])
                in_ap = (r2t[:, CHL2 - 1:T - 1]
                         .rearrange("p (j c) -> p j c", j=NCH2 - 1))
                nc.scalar.activation(out_ap, in_ap, COPY,
                                     bias=-THETA, scale=CD / VSP)

            # layer-2 spike chain; store keeps V*s (host divides by V)
            z2 = l2p.tile([H2, LAN2], bf16, tag="z2")
            q2 = l2p.tile([H2, LAN2], bf16, tag="q2")
            m2 = l2p.tile([H2, LAN2], bf16, tag="m2")
            nc.gpsimd.memset(z2[:], 0.0)
            nc.gpsimd.memset(q2[:], 0.0)
            z2v = z2[:].rearrange("p (b j) -> p b j", b=B_PER)
            q2v = q2[:].rearrange("p (b j) -> p b j", b=B_PER)
            m2v = m2[:].rearrange("p (b j) -> p b j", b=B_PER)
            for i in range(NSTEP2):
                j0, ci = _chunk_slices(i, CHL2, WARM2)
                if j0 == 0:
                    zs, qs, ms = z2[:], q2[:], m2[:]
                    us = u2[:, ci * LAN2:(ci + 1) * LAN2]
                    ss = s2[:, i * LAN2:(i + 1) * LAN2]
                else:
                    zs, qs = z2v[:, :, j0:], q2v[:, :, j0:]
                    ms = m2v[:, :, j0:]
                    # state slot j warms up on chunk j-j0's history
                    us = u25[:, ci, :, 0:NCH2 - j0]
                    ss = s25[:, i, :, j0:]
                nc.vector.scalar_tensor_tensor(zs, zs, D, qs, op0=MULT, op1=ADD)
                nc.vector.tensor_tensor(ms, zs, us, op=ADD)
                nc.vector.tensor_scalar(ss, ms, 0.0, VSP, op0=IS_GE, op1=MULT)
                nc.vector.scalar_tensor_tensor(qs, qs, D, ss, op0=MULT, op1=ADD)

            # ship the whole step-major spike store; host extracts output phase
            nc.sync.dma_start(y_d[:], s2[:])
            if DEBUG_DUMP:
                nc.sync.dma_start(s1_d[:], s_st[:])

    nc.compile()
    return nc


def _get_nc():
    if "nc" not in _CACHE:
        _CACHE["nc"] = _build()
    return _CACHE["nc"]


def _prep_inputs(downsampled, w1, w2):
    x = np.ascontiguousarray(downsampled.reshape(B_TOT, F_IN, T))
    xpad = np.zeros((B_TOT, F_PAD, T), dtype=E4M3)
    xpad[:, :F_IN] = x.astype(E4M3)          # binary spikes: exact in e4m3
    w1t = np.zeros((F_PAD, H1), dtype=E4M3)
    w1t[:F_IN] = np.ascontiguousarray(w1.T).astype(E4M3)
    # [f, o] -> [ot][kp][s][p][o_local] so one o-tile is a linear DMA
    w1t = np.ascontiguousarray(
        w1t.reshape(KP1, 2, 128, OT1, 128).transpose(3, 0, 1, 2, 4)
        .reshape(OT1, KP1 * 2 * 128 * 128))
    w2t = np.ascontiguousarray(
        w2.T.reshape(KT2, 128, H2).transpose(1, 0, 2).reshape(128, KT2 * H2)
    ).astype(BF16)
    return [
        {"x": np.ascontiguousarray(xpad[c * B_PER:(c + 1) * B_PER]),
         "w1t": w1t, "w2t": w2t}
        for c in range(N_CORES)
    ]


def kernel(downsampled: np.ndarray, w1: np.ndarray, w2: np.ndarray) -> np.ndarray:
    from concourse.bass_utils import run_bass_kernel_spmd

    nc = _get_nc()
    in_maps = _prep_inputs(downsampled, w1, w2)
    res = run_bass_kernel_spmd(nc, in_maps, core_ids=list(range(N_CORES)))
    out = np.stack([res.results[c]["y"] for c in range(N_CORES)])
    # y is the step-major spike store: [o2, (i b j)]; output phase i>=WARM2
    # holds spikes for t = j*CHL2 + (i - WARM2), scaled by V.
    out = out.reshape(N_CORES, H2, NSTEP2, B_PER, NCH2).astype(np.float32)
    out = out[:, :, WARM2:]                      # (core, o2, c, b, j)
    out = out.transpose(0, 3, 1, 4, 2)           # core, b, o2, j, c
    out = out.reshape(B_TOT, H2, T) / np.float32(VSP)   # V*s -> s (exact)
    return np.ascontiguousarray(out.astype(np.float32))


# revision 31
# speedup vs baseline: 2.3341x; 1.0613x over previous
"""Trainium2 Bass kernel for the SLAYER-style 2-layer spiking encoder.

Pipeline per core (2 batches per core, 8 cores, pure data-parallel over batch):
  fc1 (PE, fp8-e4m3 DoubleRow, k-streamed)  ->  alpha-psp scans (DVE
  tensor_tensor_scan)  ->  membrane epilogue (ACT, c-major layout)  ->
  layer-1 spike chain (DVE, 4 ops/step, both batches in one 320-lane chain,
  20 time chunks x 25 steps + 16-step warmup)  ->  fc2 (PE, strided read of
  the step-major spike store)  ->  alpha-psp scans  ->  layer-2 spike chain
  (50 chunks x 10 + 10-step warmup)  ->  DMA out (host divides by the spike
  scale to recover 0/1 spikes).

Key algebraic facts exploited:
  * alpha_psp is linear and commutes with the feature-contracting matmuls:
    matmul the raw binary spikes (exactly representable in fp8/bf16), filter
    the (T,1024) result instead of (T,6300).
  * alpha_psp = two cascaded one-pole recurrences -> two tensor_tensor_scan
    instructions per tile:  p[t] = d*p[t-1] + v[t];  r[t] = d*r[t-1] + p[t];
    membrane drive  u[t] = c*d*r[t-1] - theta.
  * spike_dyn state decays by e^-1 per step, so time chunks processed in
    parallel SIMD lanes from zero state match the sequential result after a
    short warmup (residual 2e-7 .. 4e-3 vs. decision margins; layer-2 margin
    is ~9.0 so even large perturbations cannot flip the output).
  * spike stores hold V*s with V = -20 = bf16(d*cref'): exactly the value the
    refractory state update needs (Q += V*s), exactly representable, and the
    1/V is folded into the next scan scale / host rescale.  This lets the
    threshold op be a two-scalar tensor_scalar (4x DVE mode) and the membrane
    add a pure tensor_tensor (2x DVE mode); scalar_tensor_tensor (used for
    the two state decays) has no fast mode.

Chain-step recurrence in device variables (Zt = d*Z, Q = d*P of the
reference's scaled states):
    Zt = (Zt * d) + Q          # scalar_tensor_tensor
    M  = Zt + U_step           # tensor_tensor      (U is c-major: contiguous)
    S' = (M >= 0) * V          # tensor_scalar      (written to spike store)
    Q  = (Q * d) + S'          # scalar_tensor_tensor
"""

import os
import numpy as np
import ml_dtypes

DEBUG_DUMP = bool(os.environ.get("K_DEBUG"))   # also emit layer-1 spike store

# ---------------------------------------------------------------- constants
B_TOT = 16
B_PER = 2
N_CORES = 8
T = 500
F_IN = 6300
F_PAD = 6400
H1 = 1024
H2 = 20
KP1 = F_PAD // 256    # 25 fp8 DoubleRow k-pair tiles
OT1 = H1 // 128       # 8
KT2 = H1 // 128       # 8

THETA = 10.0
D = float(np.float32(np.exp(-1.0)))
C = float(np.float32(np.e))
CD = C * D
VSP = -20.0           # stored spike value = bf16-exact d*cref (cref'=-54.3662)

WARM1 = 12
NCH1, CHL1 = 25, 20
NSTEP1 = CHL1 + WARM1         # 32
LAN1 = B_PER * OT1 * NCH1     # 400 chain lanes per partition-row
WARM2 = 10
NCH2, CHL2 = 50, 10
NSTEP2 = CHL2 + WARM2         # 20
LAN2 = B_PER * NCH2           # 100

BF16 = ml_dtypes.bfloat16
E4M3 = ml_dtypes.float8_e4m3
_CACHE = {}


def _chunk_slices(i, chl, warm):
    """(first active chunk j0, in-chunk column c) for chain step i."""
    t0 = i - warm
    j0 = 0 if t0 >= 0 else (-t0 + chl - 1) // chl
    return j0, t0 + j0 * chl


def _build():
    import concourse.bass as bass
    import concourse.bacc as bacc
    import concourse.mybir as mybir
    import concourse.tile as tile

    f32 = mybir.dt.float32
    bf16 = mybir.dt.bfloat16
    fp8 = mybir.dt.float8e4
    MULT = mybir.AluOpType.mult
    ADD = mybir.AluOpType.add
    IS_GE = mybir.AluOpType.is_ge
    COPY = mybir.ActivationFunctionType.Copy
    DROW = mybir.MatmulPerfMode.DoubleRow

    nc = bacc.Bacc("TRN2", target_bir_lowering=False, debug=False,
                   num_devices=N_CORES)

    x_d = nc.dram_tensor("x", [B_PER, F_PAD, T], fp8, kind="ExternalInput").ap()
    # host-permuted weights: [ot][kp][s][p][o] so one o-tile = one linear DMA
    w1t_d = nc.dram_tensor("w1t", [OT1, KP1 * 2 * 128 * 128], fp8,
                           kind="ExternalInput").ap()
    w2t_d = nc.dram_tensor("w2t", [128, KT2 * H2], bf16, kind="ExternalInput").ap()
    y_d = nc.dram_tensor("y", [H2, NSTEP2 * LAN2], bf16,
                         kind="ExternalOutput").ap()
    s1_d = (nc.dram_tensor("s1dump", [128, NSTEP1 * LAN1], bf16,
                           kind="ExternalOutput").ap() if DEBUG_DUMP else None)

    with tile.TileContext(nc) as tc:
        with (
            tc.tile_pool(name="xs", bufs=2) as xsp,
            tc.tile_pool(name="w1k", bufs=3) as w1kp,
            tc.tile_pool(name="wee", bufs=1) as wee,
            tc.tile_pool(name="ust", bufs=1) as ustp,
            tc.tile_pool(name="sst", bufs=1) as sstp,
            tc.tile_pool(name="scan", bufs=6) as scanp,
            tc.tile_pool(name="cst", bufs=1) as cstp,
            tc.tile_pool(name="state", bufs=3) as statep,
            tc.tile_pool(name="l2", bufs=1) as l2p,
            tc.tile_pool(name="ps", bufs=8, space="PSUM") as psp,
        ):
            dconst = cstp.tile([128, T], f32, tag="dconst")
            nc.gpsimd.memset(dconst[:], D)
            w2sb = wee.tile([128, KT2 * H2], bf16, tag="w2sb")
            nc.sync.dma_start(w2sb[:], w2t_d[:])

            # c-major membrane store: col = c*LAN1 + b*160 + g*20 + j,
            # holding U[t = j*CHL1 + c] = c*d*r[t-1] - theta  (bf16)
            u_cm = ustp.tile([128, CHL1 * LAN1], bf16, tag="ust")
            u5 = u_cm[:].rearrange("p (c b g j) -> p c b g j",
                                   c=CHL1, b=B_PER, g=OT1)
            nc.gpsimd.memset(u5[:, 0, :, :, 0], -THETA)   # t = 0
            # step-major spike store (contiguous per chain step)
            s_st = sstp.tile([128, NSTEP1 * LAN1], bf16, tag="sst")
            s5 = s_st[:].rearrange("p (i b g j) -> p i b g j",
                                   i=NSTEP1, b=B_PER, g=OT1)
            # layer-1 chain state (warmup phases run per batch, interleaved
            # with the other batch's fc1; main phase runs all 320 lanes)
            zt = statep.tile([128, LAN1], bf16, tag="state", name="z1")
            qt = statep.tile([128, LAN1], bf16, tag="state", name="q1")
            mt = statep.tile([128, LAN1], bf16, tag="state", name="m1")
            nc.gpsimd.memset(zt[:], 0.0)
            nc.gpsimd.memset(qt[:], 0.0)
            z5 = zt[:].rearrange("p (b g j) -> p b g j", b=B_PER, g=OT1)
            q5 = qt[:].rearrange("p (b g j) -> p b g j", b=B_PER, g=OT1)
            m5 = mt[:].rearrange("p (b g j) -> p b g j", b=B_PER, g=OT1)

            # ============== per-batch fc1 + scans + membrane epilogue
            # o-major: x resident per batch, one PSUM bank at a time, so each
            # o-tile's scans/epilogue pipeline inside fc1.
            # batch-0 x and the first weight tile first, so the PE can start
            # as soon as those land; batch-1 x prefetches during batch 0.
            xtiles = []
            for b in range(B_PER):
                xr = xsp.tile([128, KP1 * 2 * T], fp8, tag="xs", name=f"x_{b}")
                xtiles.append(xr[:].rearrange("p (kp s t) -> p kp s t",
                                              kp=KP1, s=2))
            xsrc = [x_d[b].rearrange("(kp s p) t -> p kp s t", s=2, p=128)
                    for b in range(B_PER)]
            for j in range(0, KP1, 5):
                nc.sync.dma_start(xtiles[0][:, j:j + 5], xsrc[0][:, j:j + 5])
            for b in range(B_PER):
                x4 = xtiles[b]
                for ot in range(OT1):
                    w1o = w1kp.tile([128, KP1 * 2 * 128], fp8, tag="w1k",
                                    name=f"w1_{b}_{ot}")
                    nc.sync.dma_start(
                        w1o[:].rearrange("p (kp s o) -> p kp s o", kp=KP1, s=2),
                        w1t_d[ot].rearrange("(kp s p o) -> p kp s o",
                                            kp=KP1, s=2, p=128))
                    if b == 0 and 1 <= ot <= 5:
                        j = (ot - 1) * 5   # batch-1 x prefetch, deprioritized
                        nc.sync.dma_start(xtiles[1][:, j:j + 5],
                                          xsrc[1][:, j:j + 5])
                    w4 = w1o[:].rearrange("p (kp s o) -> p kp s o", kp=KP1, s=2)
                    v1 = psp.tile([128, T], f32, tag="ps", name=f"v1_{b}_{ot}")
                    for kp in range(KP1):
                        nc.tensor.matmul(
                            v1[:], w4[:, kp], x4[:, kp],
                            start=(kp == 0), stop=(kp == KP1 - 1),
                            perf_mode=DROW,
                        )
                    # alpha-psp scans (DVE) + c-major membrane writes (ACT)
                    p_t = scanp.tile([128, T], f32, tag="scan", name=f"p_{b}_{ot}")
                    r_t = scanp.tile([128, T], f32, tag="scan", name=f"r_{b}_{ot}")
                    nc.vector.tensor_tensor_scan(
                        p_t[:], dconst[:], v1[:], 0.0, op0=MULT, op1=ADD)
                    nc.vector.tensor_tensor_scan(
                        r_t[:], dconst[:], p_t[:], 0.0, op0=MULT, op1=ADD)
                    # chunk j=0, c>=1:  U[t=c] <- cd*r[c-1] - th
                    nc.scalar.activation(
                        u5[:, 1:, b, ot, 0], r_t[:, 0:CHL1 - 1],
                        COPY, bias=-THETA, scale=CD)
                    # chunks j>=1, all c: U[t=j*CHL1+c] <- cd*r[t-1] - th
                    out_ap = u5[:, :, b, ot, 1:].transpose([0, 2, 1])
                    in_ap = (r_t[:, CHL1 - 1:T - 1]
                             .rearrange("p (j c) -> p j c", j=NCH1 - 1))
                    nc.scalar.activation(out_ap, in_ap, COPY,
                                         bias=-THETA, scale=CD)

                # batch-b chain warmup: b=0's fills the DVE idle window while
                # the PE runs batch 1's fc1 (touches only this batch's lanes)
                bs = slice(b, b + 1)
                for i in range(WARM1):
                    _, ci = _chunk_slices(i, CHL1, WARM1)
                    zs, qs = z5[:, bs, :, 1:], q5[:, bs, :, 1:]
                    ms = m5[:, bs, :, 1:]
                    us = u5[:, ci, bs, :, 0:NCH1 - 1]
                    ss = s5[:, i, bs, :, 1:]
                    nc.vector.scalar_tensor_tensor(zs, zs, D, qs, op0=MULT, op1=ADD)
                    nc.vector.tensor_tensor(ms, zs, us, op=ADD)
                    nc.vector.tensor_scalar(ss, ms, 0.0, VSP, op0=IS_GE, op1=MULT)
                    nc.vector.scalar_tensor_tensor(qs, qs, D, ss, op0=MULT, op1=ADD)

            # ============== layer-1 spike chain main phase (400 lanes), with
            # fc2 matmuls pipelined in 4-step groups (the PE is idle here)
            FCG = 4
            v2t = [psp.tile([H2, T], f32, tag="ps", name=f"v2_{b}")
                   for b in range(B_PER)]
            for i in range(WARM1, NSTEP1):
                ci = i - WARM1
                zs, qs, ms = zt[:], qt[:], mt[:]
                us = u_cm[:, ci * LAN1:(ci + 1) * LAN1]
                ss = s_st[:, i * LAN1:(i + 1) * LAN1]
                nc.vector.scalar_tensor_tensor(zs, zs, D, qs, op0=MULT, op1=ADD)
                nc.vector.tensor_tensor(ms, zs, us, op=ADD)
                nc.vector.tensor_scalar(ss, ms, 0.0, VSP, op0=IS_GE, op1=MULT)
                nc.vector.scalar_tensor_tensor(qs, qs, D, ss, op0=MULT, op1=ADD)
                if ci % FCG == FCG - 1:
                    for b in range(B_PER):
                        v2r = v2t[b][:].rearrange("p (j c) -> p j c", j=NCH1)
                        for kt in range(KT2):
                            rhs = (s5[:, i - FCG + 1:i + 1, b, kt, :]
                                   .transpose([0, 2, 1]))
                            nc.tensor.matmul(
                                v2r[:, :, ci - FCG + 1:ci + 1],
                                w2sb[:, kt * H2:(kt + 1) * H2],
                                rhs,
                                start=(kt == 0), stop=(kt == KT2 - 1),
                            )

            # ============== layer 2
            # c-major membrane/spikes: col = c*LAN2 + b*NCH2 + j,
            # t = j*CHL2 + c
            u2 = l2p.tile([H2, CHL2 * LAN2], bf16, tag="u2")
            s2 = l2p.tile([H2, NSTEP2 * LAN2], bf16, tag="s2")
            u25 = u2[:].rearrange("p (c b j) -> p c b j", c=CHL2, b=B_PER)
            s25 = s2[:].rearrange("p (i b j) -> p i b j", i=NSTEP2, b=B_PER)
            nc.gpsimd.memset(u25[:, 0, :, 0], -THETA)
            for b in range(B_PER):
                v2 = v2t[b]
                p2t = scanp.tile([H2, T], f32, tag="scan", name=f"p2_{b}")
                r2t = scanp.tile([H2, T], f32, tag="scan", name=f"r2_{b}")
                nc.vector.tensor_tensor_scan(
                    p2t[:], dconst[0:H2, :], v2[:], 0.0, op0=MULT, op1=ADD)
                nc.vector.tensor_tensor_scan(
                    r2t[:], dconst[0:H2, :], p2t[:], 0.0, op0=MULT, op1=ADD)
                nc.scalar.activation(
                    u25[:, 1:, b, 0], r2t[:, 0:CHL2 - 1],
                    COPY, bias=-THETA, scale=CD / VSP)
                out_ap = u25[:, :, b, 1:].transpose([0, 2, 1])
                in_ap = (r2t[:, CHL2 - 1:T - 1]
                         .rearrange("p (j c) -> p j c", j=NCH2 - 1))
                nc.scalar.activation(out_ap, in_ap, COPY,
                                     bias=-THETA, scale=CD / VSP)

            # layer-2 spike chain; store keeps V*s (host divides by V)
            z2 = l2p.tile([H2, LAN2], bf16, tag="z2")
            q2 = l2p.tile([H2, LAN2], bf16, tag="q2")
            m2 = l2p.tile([H2, LAN2], bf16, tag="m2")
            nc.gpsimd.memset(z2[:], 0.0)
            nc.gpsimd.memset(q2[:], 0.0)
            z2v = z2[:].rearrange("p (b j) -> p b j", b=B_PER)
            q2v = q2[:].rearrange("p (b j) -> p b j", b=B_PER)
            m2v = m2[:].rearrange("p (b j) -> p b j", b=B_PER)
            for i in range(NSTEP2):
                j0, ci = _chunk_slices(i, CHL2, WARM2)
                if j0 == 0:
                    zs, qs, ms = z2[:], q2[:], m2[:]
                    us = u2[:, ci * LAN2:(ci + 1) * LAN2]
                    ss = s2[:, i * LAN2:(i + 1) * LAN2]
                else:
                    zs, qs = z2v[:, :, j0:], q2v[:, :, j0:]
                    ms = m2v[:, :, j0:]
                    # state slot j warms up on chunk j-j0's history
                    us = u25[:, ci, :, 0:NCH2 - j0]
                    ss = s25[:, i, :, j0:]
                nc.vector.scalar_tensor_tensor(zs, zs, D, qs, op0=MULT, op1=ADD)
                nc.vector.tensor_tensor(ms, zs, us, op=ADD)
                nc.vector.tensor_scalar(ss, ms, 0.0, VSP, op0=IS_GE, op1=MULT)
                nc.vector.scalar_tensor_tensor(qs, qs, D, ss, op0=MULT, op1=ADD)

            # ship the whole step-major spike store; host extracts output phase
            nc.sync.dma_start(y_d[:], s2[:])
            if DEBUG_DUMP:
                nc.sync.dma_start(s1_d[:], s_st[:])

    nc.compile()
    return nc


def _get_nc():
    if "nc" not in _CACHE:
        _CACHE["nc"] = _build()
    return _CACHE["nc"]


def _prep_inputs(downsampled, w1, w2):
    x = np.ascontiguousarray(downsampled.reshape(B_TOT, F_IN, T))
    xpad = np.zeros((B_TOT, F_PAD, T), dtype=E4M3)
    xpad[:, :F_IN] = x.astype(E4M3)          # binary spikes: exact in e4m3
    w1t = np.zeros((F_PAD, H1), dtype=E4M3)
    w1t[:F_IN] = np.ascontiguousarray(w1.T).astype(E4M3)
    # [f, o] -> [ot][kp][s][p][o_local] so one o-tile is a linear DMA
    w1t = np.ascontiguousarray(
        w1t.reshape(KP1, 2, 128, OT1, 128).transpose(3, 0, 1, 2, 4)
        .reshape(OT1, KP1 * 2 * 128 * 128))
    w2t = np.ascontiguousarray(
        w2.T.reshape(KT2, 128, H2).transpose(1, 0, 2).reshape(128, KT2 * H2)
    ).astype(BF16)
    return [
        {"x": np.ascontiguousarray(xpad[c * B_PER:(c + 1) * B_PER]),
         "w1t": w1t, "w2t": w2t}
        for c in range(N_CORES)
    ]


def kernel(downsampled: np.ndarray, w1: np.ndarray, w2: np.ndarray) -> np.ndarray:
    from concourse.bass_utils import run_bass_kernel_spmd

    nc = _get_nc()
    in_maps = _prep_inputs(downsampled, w1, w2)
    res = run_bass_kernel_spmd(nc, in_maps, core_ids=list(range(N_CORES)))
    out = np.stack([res.results[c]["y"] for c in range(N_CORES)])
    # y is the step-major spike store: [o2, (i b j)]; output phase i>=WARM2
    # holds spikes for t = j*CHL2 + (i - WARM2), scaled by V.
    out = out.reshape(N_CORES, H2, NSTEP2, B_PER, NCH2).astype(np.float32)
    out = out[:, :, WARM2:]                      # (core, o2, c, b, j)
    out = out.transpose(0, 3, 1, 4, 2)           # core, b, o2, j, c
    out = out.reshape(B_TOT, H2, T) / np.float32(VSP)   # V*s -> s (exact)
    return np.ascontiguousarray(out.astype(np.float32))


# revision 35
# speedup vs baseline: 2.5120x; 1.0762x over previous
"""Trainium2 Bass kernel for the SLAYER-style 2-layer spiking encoder.

Pipeline per core (2 batches per core, 8 cores, pure data-parallel over batch):
  fc1 (PE, fp8-e4m3 DoubleRow, k-streamed)  ->  alpha-psp scans (DVE
  tensor_tensor_scan)  ->  membrane epilogue (ACT, c-major layout)  ->
  layer-1 spike chain (DVE, 4 ops/step, both batches in one 320-lane chain,
  20 time chunks x 25 steps + 16-step warmup)  ->  fc2 (PE, strided read of
  the step-major spike store)  ->  alpha-psp scans  ->  layer-2 spike chain
  (50 chunks x 10 + 10-step warmup)  ->  DMA out (host divides by the spike
  scale to recover 0/1 spikes).

Key algebraic facts exploited:
  * alpha_psp is linear and commutes with the feature-contracting matmuls:
    matmul the raw binary spikes (exactly representable in fp8/bf16), filter
    the (T,1024) result instead of (T,6300).
  * alpha_psp = two cascaded one-pole recurrences -> two tensor_tensor_scan
    instructions per tile:  p[t] = d*p[t-1] + v[t];  r[t] = d*r[t-1] + p[t];
    membrane drive  u[t] = c*d*r[t-1] - theta.
  * spike_dyn state decays by e^-1 per step, so time chunks processed in
    parallel SIMD lanes from zero state match the sequential result after a
    short warmup (residual 2e-7 .. 4e-3 vs. decision margins; layer-2 margin
    is ~9.0 so even large perturbations cannot flip the output).
  * spike stores hold V*s with V = -20 = bf16(d*cref'): exactly the value the
    refractory state update needs (Q += V*s), exactly representable, and the
    1/V is folded into the next scan scale / host rescale.  This lets the
    threshold op be a two-scalar tensor_scalar (4x DVE mode) and the membrane
    add a pure tensor_tensor (2x DVE mode); scalar_tensor_tensor (used for
    the two state decays) has no fast mode.

Chain-step recurrence in device variables (Zt = d*Z, Q = d*P of the
reference's scaled states):
    Zt = (Zt * d) + Q          # scalar_tensor_tensor
    M  = Zt + U_step           # tensor_tensor      (U is c-major: contiguous)
    S' = (M >= 0) * V          # tensor_scalar      (written to spike store)
    Q  = (Q * d) + S'          # scalar_tensor_tensor
"""

import os
import numpy as np
import ml_dtypes

DEBUG_DUMP = bool(os.environ.get("K_DEBUG"))   # also emit layer-1 spike store

# ---------------------------------------------------------------- constants
B_TOT = 16
B_PER = 2
N_CORES = 8
T = 500
F_IN = 6300
F_PAD = 6400
H1 = 1024
H2 = 20
KP1 = F_PAD // 256    # 25 fp8 DoubleRow k-pair tiles
OT1 = H1 // 128       # 8
KT2 = H1 // 128       # 8

THETA = 10.0
D = float(np.float32(np.exp(-1.0)))
C = float(np.float32(np.e))
CD = C * D
VSP = -20.0           # stored spike value = bf16-exact d*cref (cref'=-54.3662)

WARM1 = 12
NCH1, CHL1 = 25, 20
NSTEP1 = CHL1 + WARM1         # 32
LAN1 = B_PER * OT1 * NCH1     # 400 chain lanes per partition-row
WARM2 = 5
NCH2, CHL2 = 100, 5
NSTEP2 = CHL2 + WARM2         # 10
LAN2 = B_PER * NCH2           # 200

BF16 = ml_dtypes.bfloat16
E4M3 = ml_dtypes.float8_e4m3
_CACHE = {}


def _chunk_slices(i, chl, warm):
    """(first active chunk j0, in-chunk column c) for chain step i."""
    t0 = i - warm
    j0 = 0 if t0 >= 0 else (-t0 + chl - 1) // chl
    return j0, t0 + j0 * chl


def _build():
    import concourse.bass as bass
    import concourse.bacc as bacc
    import concourse.mybir as mybir
    import concourse.tile as tile

    f32 = mybir.dt.float32
    bf16 = mybir.dt.bfloat16
    fp8 = mybir.dt.float8e4
    MULT = mybir.AluOpType.mult
    ADD = mybir.AluOpType.add
    IS_GE = mybir.AluOpType.is_ge
    COPY = mybir.ActivationFunctionType.Copy
    DROW = mybir.MatmulPerfMode.DoubleRow

    nc = bacc.Bacc("TRN2", target_bir_lowering=False, debug=False,
                   num_devices=N_CORES)

    # x host-permuted to partition-major [b][p][kp][s][t]: SBUF-aligned DMA
    x_d = nc.dram_tensor("x", [B_PER, 128, KP1 * 2 * T], fp8,
                         kind="ExternalInput").ap()
    # host-permuted weights: [ot][kp][s][p][o] so one o-tile = one linear DMA
    w1t_d = nc.dram_tensor("w1t", [OT1, KP1 * 2 * 128 * 128], fp8,
                           kind="ExternalInput").ap()
    w2t_d = nc.dram_tensor("w2t", [128, KT2 * H2], bf16, kind="ExternalInput").ap()
    y_d = nc.dram_tensor("y", [H2, NSTEP2 * LAN2], bf16,
                         kind="ExternalOutput").ap()
    s1_d = (nc.dram_tensor("s1dump", [128, NSTEP1 * LAN1], bf16,
                           kind="ExternalOutput").ap() if DEBUG_DUMP else None)

    with tile.TileContext(nc) as tc:
        with (
            tc.tile_pool(name="xs", bufs=2) as xsp,
            tc.tile_pool(name="w1k", bufs=3) as w1kp,
            tc.tile_pool(name="wee", bufs=1) as wee,
            tc.tile_pool(name="ust", bufs=1) as ustp,
            tc.tile_pool(name="sst", bufs=1) as sstp,
            tc.tile_pool(name="scan", bufs=6) as scanp,
            tc.tile_pool(name="cst", bufs=1) as cstp,
            tc.tile_pool(name="state", bufs=3) as statep,
            tc.tile_pool(name="l2", bufs=1) as l2p,
            tc.tile_pool(name="ps", bufs=8, space="PSUM") as psp,
        ):
            dconst = cstp.tile([128, T], f32, tag="dconst")
            nc.gpsimd.memset(dconst[:], D)
            w2sb = wee.tile([128, KT2 * H2], bf16, tag="w2sb")
            nc.sync.dma_start(w2sb[:], w2t_d[:])

            # c-major membrane store: col = c*LAN1 + b*160 + g*20 + j,
            # holding U[t = j*CHL1 + c] = c*d*r[t-1] - theta  (bf16)
            u_cm = ustp.tile([128, CHL1 * LAN1], bf16, tag="ust")
            u5 = u_cm[:].rearrange("p (c b g j) -> p c b g j",
                                   c=CHL1, b=B_PER, g=OT1)
            nc.gpsimd.memset(u5[:, 0, :, :, 0], -THETA)   # t = 0
            # step-major spike store (contiguous per chain step)
            s_st = sstp.tile([128, NSTEP1 * LAN1], bf16, tag="sst")
            s5 = s_st[:].rearrange("p (i b g j) -> p i b g j",
                                   i=NSTEP1, b=B_PER, g=OT1)
            # layer-1 chain state (warmup phases run per batch, interleaved
            # with the other batch's fc1; main phase runs all 320 lanes)
            zt = statep.tile([128, LAN1], bf16, tag="state", name="z1")
            qt = statep.tile([128, LAN1], bf16, tag="state", name="q1")
            mt = statep.tile([128, LAN1], bf16, tag="state", name="m1")
            nc.gpsimd.memset(zt[:], 0.0)
            nc.gpsimd.memset(qt[:], 0.0)
            z5 = zt[:].rearrange("p (b g j) -> p b g j", b=B_PER, g=OT1)
            q5 = qt[:].rearrange("p (b g j) -> p b g j", b=B_PER, g=OT1)
            m5 = mt[:].rearrange("p (b g j) -> p b g j", b=B_PER, g=OT1)

            # ============== per-batch fc1 + scans + membrane epilogue
            # o-major: x resident per batch, one PSUM bank at a time, so each
            # o-tile's scans/epilogue pipeline inside fc1.
            # batch-0 x and the first weight tile first, so the PE can start
            # as soon as those land; batch-1 x prefetches during batch 0.
            xtiles = []
            for b in range(B_PER):
                xr = xsp.tile([128, KP1 * 2 * T], fp8, tag="xs", name=f"x_{b}")
                xtiles.append(xr[:].rearrange("p (kp s t) -> p kp s t",
                                              kp=KP1, s=2))
            xsrc = [x_d[b].rearrange("p (kp s t) -> p kp s t", kp=KP1, s=2)
                    for b in range(B_PER)]
            for j in range(0, KP1, 5):
                nc.sync.dma_start(xtiles[0][:, j:j + 5], xsrc[0][:, j:j + 5])
            for b in range(B_PER):
                x4 = xtiles[b]
                for ot in range(OT1):
                    w1o = w1kp.tile([128, KP1 * 2 * 128], fp8, tag="w1k",
                                    name=f"w1_{b}_{ot}")
                    nc.sync.dma_start(
                        w1o[:].rearrange("p (kp s o) -> p kp s o", kp=KP1, s=2),
                        w1t_d[ot].rearrange("(kp s p o) -> p kp s o",
                                            kp=KP1, s=2, p=128))
                    if b == 0 and 1 <= ot <= 5:
                        j = (ot - 1) * 5   # batch-1 x prefetch, deprioritized
                        nc.sync.dma_start(xtiles[1][:, j:j + 5],
                                          xsrc[1][:, j:j + 5])
                    w4 = w1o[:].rearrange("p (kp s o) -> p kp s o", kp=KP1, s=2)
                    v1 = psp.tile([128, T], f32, tag="ps", name=f"v1_{b}_{ot}")
                    for kp in range(KP1):
                        nc.tensor.matmul(
                            v1[:], w4[:, kp], x4[:, kp],
                            start=(kp == 0), stop=(kp == KP1 - 1),
                            perf_mode=DROW,
                        )
                    # alpha-psp scans (DVE) + c-major membrane writes (ACT)
                    p_t = scanp.tile([128, T], f32, tag="scan", name=f"p_{b}_{ot}")
                    r_t = scanp.tile([128, T], f32, tag="scan", name=f"r_{b}_{ot}")
                    nc.vector.tensor_tensor_scan(
                        p_t[:], dconst[:], v1[:], 0.0, op0=MULT, op1=ADD)
                    nc.vector.tensor_tensor_scan(
                        r_t[:], dconst[:], p_t[:], 0.0, op0=MULT, op1=ADD)
                    # chunk j=0, c>=1:  U[t=c] <- cd*r[c-1] - th
                    nc.scalar.activation(
                        u5[:, 1:, b, ot, 0], r_t[:, 0:CHL1 - 1],
                        COPY, bias=-THETA, scale=CD)
                    # chunks j>=1, all c: U[t=j*CHL1+c] <- cd*r[t-1] - th
                    out_ap = u5[:, :, b, ot, 1:].transpose([0, 2, 1])
                    in_ap = (r_t[:, CHL1 - 1:T - 1]
                             .rearrange("p (j c) -> p j c", j=NCH1 - 1))
                    nc.scalar.activation(out_ap, in_ap, COPY,
                                         bias=-THETA, scale=CD)

                # batch-b chain warmup: b=0's fills the DVE idle window while
                # the PE runs batch 1's fc1 (touches only this batch's lanes)
                bs = slice(b, b + 1)
                for i in range(WARM1):
                    _, ci = _chunk_slices(i, CHL1, WARM1)
                    zs, qs = z5[:, bs, :, 1:], q5[:, bs, :, 1:]
                    ms = m5[:, bs, :, 1:]
                    us = u5[:, ci, bs, :, 0:NCH1 - 1]
                    ss = s5[:, i, bs, :, 1:]
                    nc.vector.scalar_tensor_tensor(zs, zs, D, qs, op0=MULT, op1=ADD)
                    nc.vector.tensor_tensor(ms, zs, us, op=ADD)
                    nc.vector.tensor_scalar(ss, ms, 0.0, VSP, op0=IS_GE, op1=MULT)
                    nc.vector.scalar_tensor_tensor(qs, qs, D, ss, op0=MULT, op1=ADD)

            # ============== layer-1 spike chain main phase (400 lanes), with
            # fc2 matmuls pipelined in 4-step groups (the PE is idle here)
            FCG = 4
            v2t = [psp.tile([H2, T], f32, tag="ps", name=f"v2_{b}")
                   for b in range(B_PER)]
            for i in range(WARM1, NSTEP1):
                ci = i - WARM1
                zs, qs, ms = zt[:], qt[:], mt[:]
                us = u_cm[:, ci * LAN1:(ci + 1) * LAN1]
                ss = s_st[:, i * LAN1:(i + 1) * LAN1]
                nc.vector.scalar_tensor_tensor(zs, zs, D, qs, op0=MULT, op1=ADD)
                nc.vector.tensor_tensor(ms, zs, us, op=ADD)
                nc.vector.tensor_scalar(ss, ms, 0.0, VSP, op0=IS_GE, op1=MULT)
                nc.vector.scalar_tensor_tensor(qs, qs, D, ss, op0=MULT, op1=ADD)
                if ci % FCG == FCG - 1:
                    for b in range(B_PER):
                        v2r = v2t[b][:].rearrange("p (j c) -> p j c", j=NCH1)
                        for kt in range(KT2):
                            rhs = (s5[:, i - FCG + 1:i + 1, b, kt, :]
                                   .transpose([0, 2, 1]))
                            nc.tensor.matmul(
                                v2r[:, :, ci - FCG + 1:ci + 1],
                                w2sb[:, kt * H2:(kt + 1) * H2],
                                rhs,
                                start=(kt == 0), stop=(kt == KT2 - 1),
                            )

            # ============== layer 2
            # c-major membrane/spikes: col = c*LAN2 + b*NCH2 + j,
            # t = j*CHL2 + c
            u2 = l2p.tile([H2, CHL2 * LAN2], bf16, tag="u2")
            s2 = l2p.tile([H2, NSTEP2 * LAN2], bf16, tag="s2")
            u25 = u2[:].rearrange("p (c b j) -> p c b j", c=CHL2, b=B_PER)
            s25 = s2[:].rearrange("p (i b j) -> p i b j", i=NSTEP2, b=B_PER)
            nc.gpsimd.memset(u25[:, 0, :, 0], -THETA)
            for b in range(B_PER):
                v2 = v2t[b]
                p2t = scanp.tile([H2, T], f32, tag="scan", name=f"p2_{b}")
                r2t = scanp.tile([H2, T], f32, tag="scan", name=f"r2_{b}")
                nc.vector.tensor_tensor_scan(
                    p2t[:], dconst[0:H2, :], v2[:], 0.0, op0=MULT, op1=ADD)
                nc.vector.tensor_tensor_scan(
                    r2t[:], dconst[0:H2, :], p2t[:], 0.0, op0=MULT, op1=ADD)
                nc.scalar.activation(
                    u25[:, 1:, b, 0], r2t[:, 0:CHL2 - 1],
                    COPY, bias=-THETA, scale=CD / VSP)
                out_ap = u25[:, :, b, 1:].transpose([0, 2, 1])
                in_ap = (r2t[:, CHL2 - 1:T - 1]
                         .rearrange("p (j c) -> p j c", j=NCH2 - 1))
                nc.scalar.activation(out_ap, in_ap, COPY,
                                     bias=-THETA, scale=CD / VSP)

            # layer-2 spike chain; store keeps V*s (host divides by V)
            z2 = l2p.tile([H2, LAN2], bf16, tag="z2")
            q2 = l2p.tile([H2, LAN2], bf16, tag="q2")
            m2 = l2p.tile([H2, LAN2], bf16, tag="m2")
            nc.gpsimd.memset(z2[:], 0.0)
            nc.gpsimd.memset(q2[:], 0.0)
            z2v = z2[:].rearrange("p (b j) -> p b j", b=B_PER)
            q2v = q2[:].rearrange("p (b j) -> p b j", b=B_PER)
            m2v = m2[:].rearrange("p (b j) -> p b j", b=B_PER)
            for i in range(NSTEP2):
                j0, ci = _chunk_slices(i, CHL2, WARM2)
                if j0 == 0:
                    zs, qs, ms = z2[:], q2[:], m2[:]
                    us = u2[:, ci * LAN2:(ci + 1) * LAN2]
                    ss = s2[:, i * LAN2:(i + 1) * LAN2]
                else:
                    zs, qs = z2v[:, :, j0:], q2v[:, :, j0:]
                    ms = m2v[:, :, j0:]
                    # state slot j warms up on chunk j-j0's history
                    us = u25[:, ci, :, 0:NCH2 - j0]
                    ss = s25[:, i, :, j0:]
                nc.vector.scalar_tensor_tensor(zs, zs, D, qs, op0=MULT, op1=ADD)
                nc.vector.tensor_tensor(ms, zs, us, op=ADD)
                nc.vector.tensor_scalar(ss, ms, 0.0, VSP, op0=IS_GE, op1=MULT)
                nc.vector.scalar_tensor_tensor(qs, qs, D, ss, op0=MULT, op1=ADD)

            # ship the whole step-major spike store; host extracts output phase
            nc.sync.dma_start(y_d[:], s2[:])
            if DEBUG_DUMP:
                nc.sync.dma_start(s1_d[:], s_st[:])

    nc.compile()
    return nc


def _get_nc():
    if "nc" not in _CACHE:
        _CACHE["nc"] = _build()
    return _CACHE["nc"]


def _prep_inputs(downsampled, w1, w2):
    x = np.ascontiguousarray(downsampled.reshape(B_TOT, F_IN, T))
    xpad = np.zeros((B_TOT, F_PAD, T), dtype=E4M3)
    xpad[:, :F_IN] = x.astype(E4M3)          # binary spikes: exact in e4m3
    # [b, f, t] -> [b][p][kp][s][t] (partition-major, SBUF-aligned linear DMA)
    xpad = np.ascontiguousarray(
        xpad.reshape(B_TOT, KP1, 2, 128, T).transpose(0, 3, 1, 2, 4)
        .reshape(B_TOT, 128, KP1 * 2 * T))
    w1t = np.zeros((F_PAD, H1), dtype=E4M3)
    w1t[:F_IN] = np.ascontiguousarray(w1.T).astype(E4M3)
    # [f, o] -> [ot][kp][s][p][o_local] so one o-tile is a linear DMA
    w1t = np.ascontiguousarray(
        w1t.reshape(KP1, 2, 128, OT1, 128).transpose(3, 0, 1, 2, 4)
        .reshape(OT1, KP1 * 2 * 128 * 128))
    w2t = np.ascontiguousarray(
        w2.T.reshape(KT2, 128, H2).transpose(1, 0, 2).reshape(128, KT2 * H2)
    ).astype(BF16)
    return [
        {"x": np.ascontiguousarray(xpad[c * B_PER:(c + 1) * B_PER]),
         "w1t": w1t, "w2t": w2t}
        for c in range(N_CORES)
    ]


def kernel(downsampled: np.ndarray, w1: np.ndarray, w2: np.ndarray) -> np.ndarray:
    from concourse.bass_utils import run_bass_kernel_spmd

    nc = _get_nc()
    in_maps = _prep_inputs(downsampled, w1, w2)
    res = run_bass_kernel_spmd(nc, in_maps, core_ids=list(range(N_CORES)))
    out = np.stack([res.results[c]["y"] for c in range(N_CORES)])
    # y is the step-major spike store: [o2, (i b j)]; output phase i>=WARM2
    # holds spikes for t = j*CHL2 + (i - WARM2), scaled by V.
    out = out.reshape(N_CORES, H2, NSTEP2, B_PER, NCH2).astype(np.float32)
    out = out[:, :, WARM2:]                      # (core, o2, c, b, j)
    out = out.transpose(0, 3, 1, 4, 2)           # core, b, o2, j, c
    out = out.reshape(B_TOT, H2, T) / np.float32(VSP)   # V*s -> s (exact)
    return np.ascontiguousarray(out.astype(np.float32))


# revision 37
# speedup vs baseline: 2.5127x; 1.0003x over previous
"""Trainium2 Bass kernel for the SLAYER-style 2-layer spiking encoder.

Pipeline per core (2 batches per core, 8 cores, pure data-parallel over batch):
  fc1 (PE, fp8-e4m3 DoubleRow, k-streamed)  ->  alpha-psp scans (DVE
  tensor_tensor_scan)  ->  membrane epilogue (ACT, c-major layout)  ->
  layer-1 spike chain (DVE, 4 ops/step, both batches in one 320-lane chain,
  20 time chunks x 25 steps + 16-step warmup)  ->  fc2 (PE, strided read of
  the step-major spike store)  ->  alpha-psp scans  ->  layer-2 spike chain
  (50 chunks x 10 + 10-step warmup)  ->  DMA out (host divides by the spike
  scale to recover 0/1 spikes).

Key algebraic facts exploited:
  * alpha_psp is linear and commutes with the feature-contracting matmuls:
    matmul the raw binary spikes (exactly representable in fp8/bf16), filter
    the (T,1024) result instead of (T,6300).
  * alpha_psp = two cascaded one-pole recurrences -> two tensor_tensor_scan
    instructions per tile:  p[t] = d*p[t-1] + v[t];  r[t] = d*r[t-1] + p[t];
    membrane drive  u[t] = c*d*r[t-1] - theta.
  * spike_dyn state decays by e^-1 per step, so time chunks processed in
    parallel SIMD lanes from zero state match the sequential result after a
    short warmup (residual 2e-7 .. 4e-3 vs. decision margins; layer-2 margin
    is ~9.0 so even large perturbations cannot flip the output).
  * spike stores hold V*s with V = -20 = bf16(d*cref'): exactly the value the
    refractory state update needs (Q += V*s), exactly representable, and the
    1/V is folded into the next scan scale / host rescale.  This lets the
    threshold op be a two-scalar tensor_scalar (4x DVE mode) and the membrane
    add a pure tensor_tensor (2x DVE mode); scalar_tensor_tensor (used for
    the two state decays) has no fast mode.

Chain-step recurrence in device variables (Zt = d*Z, Q = d*P of the
reference's scaled states):
    Zt = (Zt * d) + Q          # scalar_tensor_tensor
    M  = Zt + U_step           # tensor_tensor      (U is c-major: contiguous)
    S' = (M >= 0) * V          # tensor_scalar      (written to spike store)
    Q  = (Q * d) + S'          # scalar_tensor_tensor
"""

import os
import numpy as np
import ml_dtypes

DEBUG_DUMP = bool(os.environ.get("K_DEBUG"))   # also emit layer-1 spike store

# ---------------------------------------------------------------- constants
B_TOT = 16
B_PER = 2
N_CORES = 8
T = 500
F_IN = 6300
F_PAD = 6400
H1 = 1024
H2 = 20
KP1 = F_PAD // 256    # 25 fp8 DoubleRow k-pair tiles
OT1 = H1 // 128       # 8
KT2 = H1 // 128       # 8

THETA = 10.0
D = float(np.float32(np.exp(-1.0)))
C = float(np.float32(np.e))
CD = C * D
VSP = -20.0           # stored spike value = bf16-exact d*cref (cref'=-54.3662)

WARM1 = 8
NCH1, CHL1 = 25, 20
NSTEP1 = CHL1 + WARM1         # 28
LAN1 = B_PER * OT1 * NCH1     # 400 chain lanes per partition-row
WARM2 = 5
NCH2, CHL2 = 100, 5
NSTEP2 = CHL2 + WARM2         # 10
LAN2 = B_PER * NCH2           # 200

BF16 = ml_dtypes.bfloat16
E4M3 = ml_dtypes.float8_e4m3
_CACHE = {}


def _chunk_slices(i, chl, warm):
    """(first active chunk j0, in-chunk column c) for chain step i."""
    t0 = i - warm
    j0 = 0 if t0 >= 0 else (-t0 + chl - 1) // chl
    return j0, t0 + j0 * chl


def _build():
    import concourse.bass as bass
    import concourse.bacc as bacc
    import concourse.mybir as mybir
    import concourse.tile as tile

    f32 = mybir.dt.float32
    bf16 = mybir.dt.bfloat16
    fp8 = mybir.dt.float8e4
    MULT = mybir.AluOpType.mult
    ADD = mybir.AluOpType.add
    IS_GE = mybir.AluOpType.is_ge
    COPY = mybir.ActivationFunctionType.Copy
    DROW = mybir.MatmulPerfMode.DoubleRow

    nc = bacc.Bacc("TRN2", target_bir_lowering=False, debug=False,
                   num_devices=N_CORES)

    # x host-permuted to partition-major [b][p][kp][s][t]: SBUF-aligned DMA
    x_d = nc.dram_tensor("x", [B_PER, 128, KP1 * 2 * T], fp8,
                         kind="ExternalInput").ap()
    # host-permuted weights: [ot][kp][s][p][o] so one o-tile = one linear DMA
    w1t_d = nc.dram_tensor("w1t", [OT1, KP1 * 2 * 128 * 128], fp8,
                           kind="ExternalInput").ap()
    w2t_d = nc.dram_tensor("w2t", [128, KT2 * H2], bf16, kind="ExternalInput").ap()
    y_d = nc.dram_tensor("y", [H2, NSTEP2 * LAN2], bf16,
                         kind="ExternalOutput").ap()
    s1_d = (nc.dram_tensor("s1dump", [128, NSTEP1 * LAN1], bf16,
                           kind="ExternalOutput").ap() if DEBUG_DUMP else None)

    with tile.TileContext(nc) as tc:
        with (
            tc.tile_pool(name="xs", bufs=2) as xsp,
            tc.tile_pool(name="w1k", bufs=3) as w1kp,
            tc.tile_pool(name="wee", bufs=1) as wee,
            tc.tile_pool(name="ust", bufs=1) as ustp,
            tc.tile_pool(name="sst", bufs=1) as sstp,
            tc.tile_pool(name="scan", bufs=6) as scanp,
            tc.tile_pool(name="cst", bufs=1) as cstp,
            tc.tile_pool(name="state", bufs=3) as statep,
            tc.tile_pool(name="l2", bufs=1) as l2p,
            tc.tile_pool(name="ps", bufs=8, space="PSUM") as psp,
        ):
            dconst = cstp.tile([128, T], f32, tag="dconst")
            nc.gpsimd.memset(dconst[:], D)
            w2sb = wee.tile([128, KT2 * H2], bf16, tag="w2sb")
            nc.sync.dma_start(w2sb[:], w2t_d[:])

            # c-major membrane store: col = c*LAN1 + b*160 + g*20 + j,
            # holding U[t = j*CHL1 + c] = c*d*r[t-1] - theta  (bf16)
            u_cm = ustp.tile([128, CHL1 * LAN1], bf16, tag="ust")
            u5 = u_cm[:].rearrange("p (c b g j) -> p c b g j",
                                   c=CHL1, b=B_PER, g=OT1)
            nc.gpsimd.memset(u5[:, 0, :, :, 0], -THETA)   # t = 0
            # step-major spike store (contiguous per chain step)
            s_st = sstp.tile([128, NSTEP1 * LAN1], bf16, tag="sst")
            s5 = s_st[:].rearrange("p (i b g j) -> p i b g j",
                                   i=NSTEP1, b=B_PER, g=OT1)
            # layer-1 chain state (warmup phases run per batch, interleaved
            # with the other batch's fc1; main phase runs all 320 lanes)
            zt = statep.tile([128, LAN1], bf16, tag="state", name="z1")
            qt = statep.tile([128, LAN1], bf16, tag="state", name="q1")
            mt = statep.tile([128, LAN1], bf16, tag="state", name="m1")
            nc.gpsimd.memset(zt[:], 0.0)
            nc.gpsimd.memset(qt[:], 0.0)
            z5 = zt[:].rearrange("p (b g j) -> p b g j", b=B_PER, g=OT1)
            q5 = qt[:].rearrange("p (b g j) -> p b g j", b=B_PER, g=OT1)
            m5 = mt[:].rearrange("p (b g j) -> p b g j", b=B_PER, g=OT1)

            # ============== per-batch fc1 + scans + membrane epilogue
            # o-major: x resident per batch, one PSUM bank at a time, so each
            # o-tile's scans/epilogue pipeline inside fc1.
            # batch-0 x and the first weight tile first, so the PE can start
            # as soon as those land; batch-1 x prefetches during batch 0.
            xtiles = []
            for b in range(B_PER):
                xr = xsp.tile([128, KP1 * 2 * T], fp8, tag="xs", name=f"x_{b}")
                xtiles.append(xr[:].rearrange("p (kp s t) -> p kp s t",
                                              kp=KP1, s=2))
            xsrc = [x_d[b].rearrange("p (kp s t) -> p kp s t", kp=KP1, s=2)
                    for b in range(B_PER)]
            for j in range(0, KP1, 5):
                nc.sync.dma_start(xtiles[0][:, j:j + 5], xsrc[0][:, j:j + 5])
            for b in range(B_PER):
                x4 = xtiles[b]
                for ot in range(OT1):
                    w1o = w1kp.tile([128, KP1 * 2 * 128], fp8, tag="w1k",
                                    name=f"w1_{b}_{ot}")
                    wv = w1o[:].rearrange("p (kp s o) -> p kp s o", kp=KP1, s=2)
                    wsrc = w1t_d[ot].rearrange("(kp s p o) -> p kp s o",
                                               kp=KP1, s=2, p=128)
                    nc.sync.dma_start(wv[:, 0:13], wsrc[:, 0:13])
                    nc.sync.dma_start(wv[:, 13:KP1], wsrc[:, 13:KP1])
                    if b == 0 and 1 <= ot <= 5:
                        j = (ot - 1) * 5   # batch-1 x prefetch, deprioritized
                        nc.sync.dma_start(xtiles[1][:, j:j + 5],
                                          xsrc[1][:, j:j + 5])
                    w4 = w1o[:].rearrange("p (kp s o) -> p kp s o", kp=KP1, s=2)
                    v1 = psp.tile([128, T], f32, tag="ps", name=f"v1_{b}_{ot}")
                    for kp in range(KP1):
                        nc.tensor.matmul(
                            v1[:], w4[:, kp], x4[:, kp],
                            start=(kp == 0), stop=(kp == KP1 - 1),
                            perf_mode=DROW,
                        )
                    # alpha-psp scans (DVE) + c-major membrane writes (ACT)
                    p_t = scanp.tile([128, T], f32, tag="scan", name=f"p_{b}_{ot}")
                    r_t = scanp.tile([128, T], f32, tag="scan", name=f"r_{b}_{ot}")
                    nc.vector.tensor_tensor_scan(
                        p_t[:], dconst[:], v1[:], 0.0, op0=MULT, op1=ADD)
                    nc.vector.tensor_tensor_scan(
                        r_t[:], dconst[:], p_t[:], 0.0, op0=MULT, op1=ADD)
                    # chunk j=0, c>=1:  U[t=c] <- cd*r[c-1] - th
                    nc.scalar.activation(
                        u5[:, 1:, b, ot, 0], r_t[:, 0:CHL1 - 1],
                        COPY, bias=-THETA, scale=CD)
                    # chunks j>=1, all c: U[t=j*CHL1+c] <- cd*r[t-1] - th
                    out_ap = u5[:, :, b, ot, 1:].transpose([0, 2, 1])
                    in_ap = (r_t[:, CHL1 - 1:T - 1]
                             .rearrange("p (j c) -> p j c", j=NCH1 - 1))
                    nc.scalar.activation(out_ap, in_ap, COPY,
                                         bias=-THETA, scale=CD)

                # batch-b chain warmup: b=0's fills the DVE idle window while
                # the PE runs batch 1's fc1 (touches only this batch's lanes)
                bs = slice(b, b + 1)
                for i in range(WARM1):
                    _, ci = _chunk_slices(i, CHL1, WARM1)
                    zs, qs = z5[:, bs, :, 1:], q5[:, bs, :, 1:]
                    ms = m5[:, bs, :, 1:]
                    us = u5[:, ci, bs, :, 0:NCH1 - 1]
                    ss = s5[:, i, bs, :, 1:]
                    nc.vector.scalar_tensor_tensor(zs, zs, D, qs, op0=MULT, op1=ADD)
                    nc.vector.tensor_tensor(ms, zs, us, op=ADD)
                    nc.vector.tensor_scalar(ss, ms, 0.0, VSP, op0=IS_GE, op1=MULT)
                    nc.vector.scalar_tensor_tensor(qs, qs, D, ss, op0=MULT, op1=ADD)

            # ============== layer-1 spike chain main phase (400 lanes), with
            # fc2 matmuls pipelined in 4-step groups (the PE is idle here)
            FCG = 4
            v2t = [psp.tile([H2, T], f32, tag="ps", name=f"v2_{b}")
                   for b in range(B_PER)]
            for i in range(WARM1, NSTEP1):
                ci = i - WARM1
                zs, qs, ms = zt[:], qt[:], mt[:]
                us = u_cm[:, ci * LAN1:(ci + 1) * LAN1]
                ss = s_st[:, i * LAN1:(i + 1) * LAN1]
                nc.vector.scalar_tensor_tensor(zs, zs, D, qs, op0=MULT, op1=ADD)
                nc.vector.tensor_tensor(ms, zs, us, op=ADD)
                nc.vector.tensor_scalar(ss, ms, 0.0, VSP, op0=IS_GE, op1=MULT)
                nc.vector.scalar_tensor_tensor(qs, qs, D, ss, op0=MULT, op1=ADD)
                if ci % FCG == FCG - 1:
                    for b in range(B_PER):
                        v2r = v2t[b][:].rearrange("p (j c) -> p j c", j=NCH1)
                        for kt in range(KT2):
                            rhs = (s5[:, i - FCG + 1:i + 1, b, kt, :]
                                   .transpose([0, 2, 1]))
                            nc.tensor.matmul(
                                v2r[:, :, ci - FCG + 1:ci + 1],
                                w2sb[:, kt * H2:(kt + 1) * H2],
                                rhs,
                                start=(kt == 0), stop=(kt == KT2 - 1),
                            )

            # ============== layer 2
            # c-major membrane/spikes: col = c*LAN2 + b*NCH2 + j,
            # t = j*CHL2 + c
            u2 = l2p.tile([H2, CHL2 * LAN2], bf16, tag="u2")
            s2 = l2p.tile([H2, NSTEP2 * LAN2], bf16, tag="s2")
            u25 = u2[:].rearrange("p (c b j) -> p c b j", c=CHL2, b=B_PER)
            s25 = s2[:].rearrange("p (i b j) -> p i b j", i=NSTEP2, b=B_PER)
            nc.gpsimd.memset(u25[:, 0, :, 0], -THETA)
            for b in range(B_PER):
                v2 = v2t[b]
                p2t = scanp.tile([H2, T], f32, tag="scan", name=f"p2_{b}")
                r2t = scanp.tile([H2, T], f32, tag="scan", name=f"r2_{b}")
                nc.vector.tensor_tensor_scan(
                    p2t[:], dconst[0:H2, :], v2[:], 0.0, op0=MULT, op1=ADD)
                nc.vector.tensor_tensor_scan(
                    r2t[:], dconst[0:H2, :], p2t[:], 0.0, op0=MULT, op1=ADD)
                nc.scalar.activation(
                    u25[:, 1:, b, 0], r2t[:, 0:CHL2 - 1],
                    COPY, bias=-THETA, scale=CD / VSP)
                out_ap = u25[:, :, b, 1:].transpose([0, 2, 1])
                in_ap = (r2t[:, CHL2 - 1:T - 1]
                         .rearrange("p (j c) -> p j c", j=NCH2 - 1))
                nc.scalar.activation(out_ap, in_ap, COPY,
                                     bias=-THETA, scale=CD / VSP)

            # layer-2 spike chain; store keeps V*s (host divides by V)
            z2 = l2p.tile([H2, LAN2], bf16, tag="z2")
            q2 = l2p.tile([H2, LAN2], bf16, tag="q2")
            m2 = l2p.tile([H2, LAN2], bf16, tag="m2")
            nc.gpsimd.memset(z2[:], 0.0)
            nc.gpsimd.memset(q2[:], 0.0)
            z2v = z2[:].rearrange("p (b j) -> p b j", b=B_PER)
            q2v = q2[:].rearrange("p (b j) -> p b j", b=B_PER)
            m2v = m2[:].rearrange("p (b j) -> p b j", b=B_PER)
            for i in range(NSTEP2):
                j0, ci = _chunk_slices(i, CHL2, WARM2)
                if j0 == 0:
                    zs, qs, ms = z2[:], q2[:], m2[:]
                    us = u2[:, ci * LAN2:(ci + 1) * LAN2]
                    ss = s2[:, i * LAN2:(i + 1) * LAN2]
                else:
                    zs, qs = z2v[:, :, j0:], q2v[:, :, j0:]
                    ms = m2v[:, :, j0:]
                    # state slot j warms up on chunk j-j0's history
                    us = u25[:, ci, :, 0:NCH2 - j0]
                    ss = s25[:, i, :, j0:]
                nc.vector.scalar_tensor_tensor(zs, zs, D, qs, op0=MULT, op1=ADD)
                nc.vector.tensor_tensor(ms, zs, us, op=ADD)
                nc.vector.tensor_scalar(ss, ms, 0.0, VSP, op0=IS_GE, op1=MULT)
                nc.vector.scalar_tensor_tensor(qs, qs, D, ss, op0=MULT, op1=ADD)

            # ship the whole step-major spike store; host extracts output phase
            nc.sync.dma_start(y_d[:], s2[:])
            if DEBUG_DUMP:
                nc.sync.dma_start(s1_d[:], s_st[:])

    nc.compile()
    return nc


def _get_nc():
    if "nc" not in _CACHE:
        _CACHE["nc"] = _build()
    return _CACHE["nc"]


def _prep_inputs(downsampled, w1, w2):
    x = np.ascontiguousarray(downsampled.reshape(B_TOT, F_IN, T))
    xpad = np.zeros((B_TOT, F_PAD, T), dtype=E4M3)
    xpad[:, :F_IN] = x.astype(E4M3)          # binary spikes: exact in e4m3
    # [b, f, t] -> [b][p][kp][s][t] (partition-major, SBUF-aligned linear DMA)
    xpad = np.ascontiguousarray(
        xpad.reshape(B_TOT, KP1, 2, 128, T).transpose(0, 3, 1, 2, 4)
        .reshape(B_TOT, 128, KP1 * 2 * T))
    w1t = np.zeros((F_PAD, H1), dtype=E4M3)
    w1t[:F_IN] = np.ascontiguousarray(w1.T).astype(E4M3)
    # [f, o] -> [ot][kp][s][p][o_local] so one o-tile is a linear DMA
    w1t = np.ascontiguousarray(
        w1t.reshape(KP1, 2, 128, OT1, 128).transpose(3, 0, 1, 2, 4)
        .reshape(OT1, KP1 * 2 * 128 * 128))
    w2t = np.ascontiguousarray(
        w2.T.reshape(KT2, 128, H2).transpose(1, 0, 2).reshape(128, KT2 * H2)
    ).astype(BF16)
    return [
        {"x": np.ascontiguousarray(xpad[c * B_PER:(c + 1) * B_PER]),
         "w1t": w1t, "w2t": w2t}
        for c in range(N_CORES)
    ]


def kernel(downsampled: np.ndarray, w1: np.ndarray, w2: np.ndarray) -> np.ndarray:
    from concourse.bass_utils import run_bass_kernel_spmd

    nc = _get_nc()
    in_maps = _prep_inputs(downsampled, w1, w2)
    res = run_bass_kernel_spmd(nc, in_maps, core_ids=list(range(N_CORES)))
    out = np.stack([res.results[c]["y"] for c in range(N_CORES)])
    # y is the step-major spike store: [o2, (i b j)]; output phase i>=WARM2
    # holds spikes for t = j*CHL2 + (i - WARM2), scaled by V.
    out = out.reshape(N_CORES, H2, NSTEP2, B_PER, NCH2).astype(np.float32)
    out = out[:, :, WARM2:]                      # (core, o2, c, b, j)
    out = out.transpose(0, 3, 1, 4, 2)           # core, b, o2, j, c
    out = out.reshape(B_TOT, H2, T) / np.float32(VSP)   # V*s -> s (exact)
    return np.ascontiguousarray(out.astype(np.float32))
